# revision 1
# baseline (speedup 1.0000x reference)
"""GQA forward kernel for Trainium2, 8-core tensor-parallel (group-aligned).

Problem: B=2, T=2048, D=2048, 32 Q heads / 8 KV heads, head_dim 64, causal.

Sharding: core c owns KV head c and its 4 Q heads (whole GQA group), both
batches.  Output projection is row-parallel Megatron style: each core
contracts its 256 attention-output channels against its slice of Wo and the
host sums the 8 partial outputs (+ bo).

Device-side dataflow per core (matmuls in float32r unless noted, fp32 accum):
  x^T [C, T] (host-transposed)
    -> QKK proj:  lhsT = [Wq_c | Wk_c | Wk_c]  -> Q^T [256, T], K^T dup [128, T]
    -> V proj (fp16): lhsT = x^T fp16, rhs = Wv_c fp16 -> V [T, 64] natural
  attention per (batch, head-pair, q-chunk of 512), scores TRANSPOSED:
    S^T[kv, q] = matmul(lhsT=K^T tile [64,128], rhs=Q^T [64, 512])
      head pairs run on disjoint PE row groups (base partitions 0 / 64)
    expS = ACT Exp(S^T / 8)  (no max-subtraction: |scores| <= ~6)
    causal: column-sliced matmuls + one triangle mask on diagonal tiles
    AV: matmul(lhsT=V2 [kv,65] (V plus ones col), rhs=expS) accumulated over
        kv tiles -> [attn^T; den] in PSUM
    normalize: den replicated to 64 partitions via K=1 ones-matmul,
        reciprocal + multiply on DVE
  out-proj: y[t, e] = matmul(lhsT=attn^T [256, t], rhs=Wo_c [256, e])
"""

import os

import numpy as np

import concourse.mybir as mybir
import concourse.tile as tile
from concourse import bacc
from concourse import bass_utils

P = 128
B = 2
T = 2048
C = 2048
HD = 64
QH = 32
KVH = 8
G = QH // KVH  # 4
NCORES = 8
QH_LOC = QH // NCORES  # 4 q heads per core
TCH = 256  # token chunk for projection phase
QCH = 512  # q chunk for attention phase
KT = C // P  # 16 contraction tiles
f32 = mybir.dt.float32
f32r = mybir.dt.float32r
bf16 = mybir.dt.bfloat16
fp16 = mybir.dt.float16

_CACHE = {}


def _build():
    nc = bacc.Bacc("TRN2", target_bir_lowering=False, debug=False, num_devices=NCORES)

    xt = nc.dram_tensor("xt", [B, C, T], f32, kind="ExternalInput")
    xtb = nc.dram_tensor("xtb", [B, C, T], fp16, kind="ExternalInput")
    wqk = nc.dram_tensor("wqk", [C, 384], f32, kind="ExternalInput")
    wv = nc.dram_tensor("wv", [C, HD], fp16, kind="ExternalInput")
    wo = nc.dram_tensor("wo", [G * HD, C], f32, kind="ExternalInput")
    bqk = nc.dram_tensor("bqk", [P, 3], f32, kind="ExternalInput")
    bv = nc.dram_tensor("bv", [1, HD], f32, kind="ExternalInput")
    maskd = nc.dram_tensor("mask", [P, P], f32, kind="ExternalInput")
    y = nc.dram_tensor("y", [B, T, C], f32, kind="ExternalOutput")

    wqk3 = wqk.ap().rearrange("(ko p) m -> p ko m", p=P).bitcast(f32r)
    wv3 = wv.ap().rearrange("(ko p) m -> p ko m", p=P)
    wo3 = wo.ap().rearrange("(ko p) m -> p ko m", p=P).bitcast(f32r)

    with tile.TileContext(nc) as tc:
        with (
            tc.tile_pool(name="const", bufs=1) as cpool,
            tc.tile_pool(name="x", bufs=2) as xpool,
            tc.tile_pool(name="proj", bufs=1) as projpool,
            tc.tile_pool(name="attn", bufs=1) as apool,
            tc.tile_pool(name="work", bufs=5) as wpool,
            tc.tile_pool(name="work2", bufs=6) as wpool2,
            tc.tile_pool(name="psA", bufs=2, space="PSUM") as psumA,
            tc.tile_pool(name="psB", bufs=2, space="PSUM") as psumB,
            tc.tile_pool(name="psC", bufs=2, space="PSUM") as psumC,
        ):
            # ---- constants / weights (resident) ----
            # startup-critical DMA order: wqk sub0, then x chunk 0 (the first
            # 16 QKK matmuls need only these), then the rest
            wqk_sb = cpool.tile([P, KT, 384], f32r)
            nc.sync.dma_start(wqk_sb[:, :, 0:P], wqk3[:, :, 0:P])
            xb0 = xt.ap()[0].rearrange("(ko p) t -> p ko t", p=P).bitcast(f32r)
            xbb0 = xtb.ap()[0].rearrange("(ko p) t -> p ko t", p=P)
            xch0 = xpool.tile([P, KT, TCH], f32r, tag="xch", name="xch")
            nc.sync.dma_start(xch0[:, 0 : KT // 2, :], xb0[:, 0 : KT // 2, 0:TCH])
            nc.sync.dma_start(xch0[:, KT // 2 :, :], xb0[:, KT // 2 :, 0:TCH])
            for _s in range(1, 3):
                nc.sync.dma_start(
                    wqk_sb[:, :, _s * P : (_s + 1) * P], wqk3[:, :, _s * P : (_s + 1) * P]
                )
            xchb0 = xpool.tile([P, KT, TCH], fp16, tag="xchb", name="xchb")
            nc.sync.dma_start(xchb0[:, 0 : KT // 2, :], xbb0[:, 0 : KT // 2, 0:TCH])
            nc.sync.dma_start(xchb0[:, KT // 2 :, :], xbb0[:, KT // 2 :, 0:TCH])
            wv_sb = cpool.tile([P, KT, HD], fp16)
            nc.sync.dma_start(wv_sb[:], wv3)
            bqk_sb = cpool.tile([P, 3], f32)
            nc.sync.dma_start(bqk_sb[:], bqk.ap())
            bv_sb = cpool.tile([P, HD], f32)
            nc.sync.dma_start(bv_sb[:], bv.ap().to_broadcast((P, HD)))
            mask_sb = cpool.tile([P, P], f32r)
            nc.sync.dma_start(mask_sb[:], maskd.ap().bitcast(f32r))
            ones_f32 = cpool.tile([P, KT], f32)
            nc.gpsimd.memset(ones_f32[:], 1.0)
            ones_r = cpool.tile([P, HD], f32r)
            nc.vector.tensor_copy(ones_r[:], ones_f32[:, 0:1].to_broadcast((P, HD)))
            wo_sb = cpool.tile([P, 2, C], f32r)

            def emit_p3(pb, pattn, pqc):
                for ts in range(pqc * (QCH // P), (pqc + 1) * (QCH // P)):
                    for ec in range(C // QCH):
                        py = psumC.tile([P, QCH], f32, tag="pp", name="py")
                        for ks in range(2):
                            nc.tensor.matmul(
                                py[:],
                                pattn[:, ks, ts * P : (ts + 1) * P],
                                wo_sb[:, ks, ec * QCH : (ec + 1) * QCH],
                                start=(ks == 0),
                                stop=(ks == 1),
                            )
                        y_sb = wpool2.tile([P, QCH], f32, tag="ysb")
                        nc.any.tensor_copy(y_sb[:], py[:])
                        nc.sync.dma_start(
                            y.ap()[
                                pb, ts * P : (ts + 1) * P, ec * QCH : (ec + 1) * QCH
                            ],
                            y_sb[:],
                        )

            deferred_p3 = None
            for b in range(B):
                xb = xt.ap()[b].rearrange("(ko p) t -> p ko t", p=P).bitcast(f32r)
                xbb = xtb.ap()[b].rearrange("(ko p) t -> p ko t", p=P)

                # ---- P1: projections ----
                qkk_sb = projpool.tile([P, 3, T], f32r, tag="qkk")
                v2_sb = projpool.tile([P, KT, 130], f32r, tag="v2")
                nc.vector.tensor_copy(v2_sb[:, :, 64:65], ones_f32[:, :, None])
                for tch in range(T // TCH):
                    tsl = slice(tch * TCH, (tch + 1) * TCH)
                    if b == 0 and tch == 0:
                        xch, xchb = xch0, xchb0
                    else:
                        xch = xpool.tile([P, KT, TCH], f32r, tag="xch", name="xch")
                        nc.sync.dma_start(xch[:, 0 : KT // 2, :], xb[:, 0 : KT // 2, tsl])
                        nc.sync.dma_start(xch[:, KT // 2 :, :], xb[:, KT // 2 :, tsl])
                        xchb = xpool.tile([P, KT, TCH], fp16, tag="xchb", name="xchb")
                        nc.sync.dma_start(xchb[:, 0 : KT // 2, :], xbb[:, 0 : KT // 2, tsl])
                        nc.sync.dma_start(xchb[:, KT // 2 :, :], xbb[:, KT // 2 :, tsl])
                    if tch == 4 and b == 0:
                        nc.sync.dma_start(wo_sb[:], wo3)
                    if tch == 3 and deferred_p3 is not None:
                        emit_p3(*deferred_p3)
                        deferred_p3 = None
                    for sub in range(3):
                        pp_full = psumC.tile([P, QCH], f32, tag="pp", name="pp")
                        pp = pp_full[:, :TCH]
                        for k in range(KT):
                            nc.tensor.matmul(
                                pp[:],
                                wqk_sb[:, k, sub * P : (sub + 1) * P],
                                xch[:, k, :],
                                start=(k == 0),
                                stop=(k == KT - 1),
                            )
                        nc.any.tensor_tensor(
                            qkk_sb[:, sub, tsl],
                            pp[:],
                            bqk_sb[:, sub : sub + 1].to_broadcast((P, TCH)),
                            mybir.AluOpType.add,
                        )
                    for ts in range(TCH // P):
                        tidx = tch * (TCH // P) + ts
                        pv = psumC.tile([P, HD], f32, tag="pp", name="pv")
                        for k in range(KT):
                            nc.tensor.matmul(
                                pv[:],
                                xchb[:, k, ts * P : (ts + 1) * P],
                                wv_sb[:, k, :],
                                start=(k == 0),
                                stop=(k == KT - 1),
                            )
                        nc.any.tensor_tensor(
                            v2_sb[:, tidx, 0:64], pv[:], bv_sb[:], mybir.AluOpType.add
                        )
                        nc.any.tensor_tensor(
                            v2_sb[:, tidx, 65:129], pv[:], bv_sb[:], mybir.AluOpType.add
                        )

                # ---- P2 + P3 interleaved: attention then out-proj per q-chunk ----
                # Head pairs (2*sub, 2*sub+1) run QK^T on disjoint PE row
                # groups (base partitions 0 / 64); their score tiles share one
                # 2-bank PSUM tile so exp is a single wide ACT op.
                attn_sb = apool.tile([P, 2, T], f32r, tag="attn")
                for qc in range(T // QCH):
                    q0 = qc * QCH
                    nfull = q0 // P
                    ntiles = nfull + QCH // P
                    for sub in range(2):
                        qT0 = qkk_sb[0:64, sub, q0 : q0 + QCH]
                        qT1 = qkk_sb[64:128, sub, q0 : q0 + QCH]
                        pav0 = psumB.tile([P, QCH], f32, tag="pav", name="pav0")
                        pav1 = psumB.tile([P, QCH], f32, tag="pav", name="pav1")
                        for i in range(ntiles):
                            if i < nfull:
                                nsl = slice(0, QCH)
                            else:
                                nsl = slice((i - nfull) * P, QCH)
                            ksl = slice(i * P, (i + 1) * P)
                            ps_s = psumA.tile([P, 2, QCH], f32, tag="ps", name="ps_s")
                            # concurrent pair: disjoint PE row groups 0-63 / 64-127
                            nc.tensor.matmul(
                                ps_s[:, 0, nsl],
                                qkk_sb[0:64, 2, ksl],
                                qT0[:, nsl],
                                start=True,
                                stop=True,
                            )
                            nc.tensor.matmul(
                                ps_s[:, 1, nsl],
                                qkk_sb[64:128, 2, ksl],
                                qT1[:, nsl],
                                start=True,
                                stop=True,
                            )
                            expS = wpool.tile([P, 2, QCH], f32r, tag="expS")
                            nc.scalar.activation(
                                expS[:, :, nsl],
                                ps_s[:, :, nsl],
                                mybir.ActivationFunctionType.Exp,
                                scale=0.125,
                            )
                            if i >= nfull:
                                j = i - nfull
                                nc.any.tensor_tensor(
                                    expS[:, :, j * P : (j + 1) * P],
                                    expS[:, :, j * P : (j + 1) * P],
                                    mask_sb[:, None, :].to_broadcast((P, 2, P)),
                                    mybir.AluOpType.mult,
                                )
                            for half, pav in ((0, pav0), (1, pav1)):
                                nc.tensor.matmul(
                                    pav[0:65, nsl],
                                    v2_sb[:, i, 0:65],
                                    expS[:, half, nsl],
                                    start=(i == 0),
                                    stop=(i == ntiles - 1),
                                    skip_group_check=True,
                                )
                        for half, pav in ((0, pav0), (1, pav1)):
                            den_sb = wpool2.tile([P, QCH], f32r, tag="den")
                            nc.any.tensor_copy(den_sb[64:65, :], pav[64:65, :])
                            ps_den = psumA.tile([64, QCH], f32, tag="ps", name="psd")
                            nc.tensor.matmul(
                                ps_den[:],
                                ones_r[64:65, 0:64],
                                den_sb[64:65, :],
                                start=True,
                                stop=True,
                            )
                            rec = wpool2.tile([64, QCH], f32, tag="rec")
                            nc.vector.reciprocal(rec[:], ps_den[:])
                            if half == 0:
                                nc.any.tensor_tensor(
                                    attn_sb[0:64, sub, q0 : q0 + QCH],
                                    pav[0:64, :],
                                    rec[:],
                                    mybir.AluOpType.mult,
                                )
                            else:
                                alo = wpool2.tile([64, QCH], f32r, tag="alo")
                                nc.any.tensor_tensor(
                                    alo[:], pav[0:64, :], rec[:], mybir.AluOpType.mult
                                )
                                nc.sync.dma_start(
                                    attn_sb[64:128, sub, q0 : q0 + QCH], alo[:]
                                )

                    # out-proj for the finished token range; the last q-chunk is
                    # deferred into the next batch's P1 (fills PE during DMA waits)
                    if qc < T // QCH - 1 or b == B - 1:
                        emit_p3(b, attn_sb, qc)
                    else:
                        deferred_p3 = (b, attn_sb, qc)

            if deferred_p3 is not None:
                emit_p3(*deferred_p3)

    nc.compile()
    return nc


def _prep_inputs(x, Wq, bq, Wk, bk, Wv, bv, Wo, bo):
    x = np.ascontiguousarray(np.asarray(x, dtype=np.float32))
    xt = np.ascontiguousarray(x.transpose(0, 2, 1))
    xtb = xt.astype(np.float16)
    Wq = np.asarray(Wq, dtype=np.float32)
    Wk = np.asarray(Wk, dtype=np.float32)
    Wv = np.asarray(Wv, dtype=np.float32)
    Wo = np.asarray(Wo, dtype=np.float32)
    bq = np.asarray(bq, dtype=np.float32)
    bk = np.asarray(bk, dtype=np.float32)
    bv = np.asarray(bv, dtype=np.float32)

    # mask[kj, qi] = 1 iff kj <= qi  (upper triangular incl. diag)
    mask = np.triu(np.ones((P, P), dtype=np.float32)).copy()
    in_maps = []
    for c in range(NCORES):
        qs = slice(c * G * HD, (c + 1) * G * HD)
        ks = slice(c * HD, (c + 1) * HD)
        wqk_c = np.concatenate([Wq[:, qs], Wk[:, ks], Wk[:, ks]], axis=1)
        bq_c = bq[qs]
        bqk_c = np.stack(
            [bq_c[0:128], bq_c[128:256], np.concatenate([bk[ks], bk[ks]])], axis=1
        )
        in_maps.append(
            {
                "xt": xt,
                "xtb": xtb,
                "wqk": np.ascontiguousarray(wqk_c),
                "wv": np.ascontiguousarray(Wv[:, ks]).astype(np.float16),
                "wo": np.ascontiguousarray(Wo[qs, :]),
                "bqk": np.ascontiguousarray(bqk_c),
                "bv": np.ascontiguousarray(bv[None, ks]),
                "mask": mask,
            }
        )
    return in_maps


def kernel(x, Wq, bq, Wk, bk, Wv, bv, Wo, bo, _trace=False):
    # NTFF tracing is unavailable through this axon client; make sure a
    # stray BASS_TRACE=1 in the environment cannot divert the run path.
    if not _trace:
        os.environ["BASS_NEVER_TRACE"] = "1"
    if "nc" not in _CACHE:
        _CACHE["nc"] = _build()
    nc = _CACHE["nc"]
    in_maps = _prep_inputs(x, Wq, bq, Wk, bk, Wv, bv, Wo, bo)
    res = bass_utils.run_bass_kernel_spmd(
        nc, in_maps, core_ids=list(range(NCORES)), trace=_trace
    )
    bo = np.asarray(bo, dtype=np.float32)
    y = np.zeros((B, T, C), dtype=np.float32)
    for c in range(NCORES):
        y += res.results[c]["y"]
    y += bo
    if _trace:
        return y, res
    return y



# revision 19
# speedup vs baseline: 1.2366x; 1.2366x over previous
"""GQA forward kernel for Trainium2, 8-core tensor-parallel (group-aligned).

Problem: B=2, T=2048, D=2048, 32 Q heads / 8 KV heads, head_dim 64, causal.

Sharding: core c owns KV head c and its 4 Q heads (whole GQA group), both
batches.  Output projection is row-parallel Megatron style: each core
contracts its 256 attention-output channels against its slice of Wo and the
host sums the 8 partial outputs (+ bo).

All device dataflow is fp16 (fp32 PSUM accumulation), which halves HBM
traffic vs fp32 and runs matmuls at 1 row/cycle at any tile width.

Per-core dataflow:
  x^T [C, T] fp16 (host-transposed)
    -> QKK proj: lhsT = [Wq_p0 | Wq_p1 | Wk | Wk] -> Q^T [256, T], K^T dup [128, T]
    -> V proj: natural orientation -> V2 [T, 65] (V plus ones col for the
       softmax denominator), per 128-token tile
  attention per (batch, q-chunk of 512):
    S^T[kv, q] = matmul(lhsT=K^T tile [64,128], rhs=Q^T [64, nsl]); the two
      heads of a pair run on disjoint PE row groups (base partitions 0 / 64)
    expS = ACT Exp(S^T / 8) -> SBUF fp16  (no max-subtraction: |S/8| <= ~6)
    causal: column-sliced matmuls + one triangle mask-mult on diagonal tiles
    AV in NATURAL orientation (half the PE cost of the transposed form):
      pav[q-tile, head, 0:65] += matmul(lhsT=expS[kv, q-tile], rhs=V2[kv, 0:65])
      accumulated over kv tiles; col 64 is the denominator.
    normalize on DVE (reciprocal + mult) -> attn [q, 256] fp16
    attn^T via PE transpose -> attnT [ch, q] (lhsT layout for out-proj)
  out-proj: y[t, e] = matmul(lhsT=attnT [256, t], rhs=Wo_c [256, e]) -> fp16

Emission is software-pipelined by hand because every engine executes its
queue in order: scores(i+1) is emitted before AV(i), out-proj blocks of the
previous q-chunk and the NEXT batch's projection chunks are spread into the
ACT-bound attention steps so the PE never parks on the exp it is about to
consume.
"""

import os

import numpy as np

import concourse.mybir as mybir
import concourse.tile as tile
from concourse import bacc
from concourse import bass_utils

P = 128
B = 2
T = 2048
C = 2048
HD = 64
QH = 32
KVH = 8
G = QH // KVH  # 4
NCORES = 8
TCH = 256  # token chunk for projection phase
QCH = 512  # q chunk for attention phase
KT = C // P  # 16 contraction tiles
NTCH = T // TCH  # 8
NQC = T // QCH  # 4
f32 = mybir.dt.float32
fp16 = mybir.dt.float16

_CACHE = {}


def _build():
    nc = bacc.Bacc("TRN2", target_bir_lowering=False, debug=False, num_devices=NCORES)

    xt = nc.dram_tensor("xt", [B, C, T], fp16, kind="ExternalInput")
    wqk = nc.dram_tensor("wqk", [C, 384], fp16, kind="ExternalInput")
    wv = nc.dram_tensor("wv", [C, HD], fp16, kind="ExternalInput")
    wo = nc.dram_tensor("wo", [G * HD, C], fp16, kind="ExternalInput")
    bqk = nc.dram_tensor("bqk", [P, 3], f32, kind="ExternalInput")
    bv = nc.dram_tensor("bv", [1, HD], f32, kind="ExternalInput")
    maskd = nc.dram_tensor("mask", [P, P], fp16, kind="ExternalInput")
    identd = nc.dram_tensor("ident", [P, P], fp16, kind="ExternalInput")
    y = nc.dram_tensor("y", [B, T, C], fp16, kind="ExternalOutput")

    wqk3 = wqk.ap().rearrange("(ko p) m -> p ko m", p=P)
    wv3 = wv.ap().rearrange("(ko p) m -> p ko m", p=P)
    wo3 = wo.ap().rearrange("(ko p) m -> p ko m", p=P)

    with tile.TileContext(nc) as tc:
        with (
            tc.tile_pool(name="const", bufs=1) as cpool,
            tc.tile_pool(name="x", bufs=3) as xpool,
            tc.tile_pool(name="proj", bufs=2) as projpool,
            tc.tile_pool(name="v2p", bufs=2) as v2pool,
            tc.tile_pool(name="exps", bufs=24) as expool,
            tc.tile_pool(name="attnT", bufs=3) as apool,
            tc.tile_pool(name="attn", bufs=4) as aspool,
            tc.tile_pool(name="y", bufs=4) as ypool,
            tc.tile_pool(name="psS", bufs=2, space="PSUM") as psumS,
            tc.tile_pool(name="psAV", bufs=2, space="PSUM") as psumAV,
            tc.tile_pool(name="psY", bufs=2, space="PSUM") as psumY,
        ):
            # ---- constants / weights (resident) ----
            # startup-critical DMA order: first half of wqk sub0 + x chunk 0
            # (the first 8 QKK matmuls need only these), then the rest
            wqk_sb = cpool.tile([P, KT, 384], fp16)
            xb0 = xt.ap()[0].rearrange("(ko p) t -> p ko t", p=P)
            xch0 = xpool.tile([P, KT, TCH], fp16, tag="xch", name="xch")
            nc.sync.dma_start(wqk_sb[:, 0 : KT // 2, 0:P], wqk3[:, 0 : KT // 2, 0:P])
            nc.sync.dma_start(xch0[:, 0 : KT // 2, :], xb0[:, 0 : KT // 2, 0:TCH])
            nc.sync.dma_start(wqk_sb[:, KT // 2 :, 0:P], wqk3[:, KT // 2 :, 0:P])
            nc.sync.dma_start(xch0[:, KT // 2 :, :], xb0[:, KT // 2 :, 0:TCH])
            xch1 = xpool.tile([P, KT, TCH], fp16, tag="xch", name="xch")
            nc.sync.dma_start(xch1[:, 0 : KT // 2, :], xb0[:, 0 : KT // 2, TCH : 2 * TCH])
            nc.sync.dma_start(xch1[:, KT // 2 :, :], xb0[:, KT // 2 :, TCH : 2 * TCH])
            for _s in range(1, 3):
                nc.sync.dma_start(
                    wqk_sb[:, :, _s * P : (_s + 1) * P], wqk3[:, :, _s * P : (_s + 1) * P]
                )
            wv_sb = cpool.tile([P, KT, HD], fp16)
            nc.sync.dma_start(wv_sb[:], wv3)
            bqk_sb = cpool.tile([P, 3], f32)
            nc.sync.dma_start(bqk_sb[:], bqk.ap())
            bv_sb = cpool.tile([P, HD], f32)
            nc.sync.dma_start(bv_sb[:], bv.ap().to_broadcast((P, HD)))
            mask_sb = cpool.tile([P, P], fp16)
            nc.sync.dma_start(mask_sb[:], maskd.ap())
            ident_sb = cpool.tile([P, P], fp16)
            nc.sync.dma_start(ident_sb[:], identd.ap())
            wo_sb = cpool.tile([P, 2, C], fp16)

            # ---- deferred-work queues (fill PE during ACT-bound stretches) --
            p3_queue = []  # out-proj (ts, ec) blocks of finished q-chunks
            p1_queue = []  # next batch's projection chunks
            p3_state = {}

            def p3_block(pb, pattnT, tglob, ec):
                # one y row-block [128, C] per q-tile: 4 (ec) matmul+copy
                # units sharing a y_sb row, one big DMA after the last ec
                def emit():
                    if ec == 0:
                        p3_state[ts_key] = ypool.tile(
                            [P, C], fp16, tag="ysb", name="ysb"
                        )
                    y_sb = p3_state[ts_key]
                    py = psumY.tile([P, QCH], f32, tag="py", name="py")
                    for ks in range(2):
                        nc.tensor.matmul(
                            py[:],
                            pattnT[:, ks, (tglob % 4) * P : (tglob % 4 + 1) * P],
                            wo_sb[:, ks, ec * QCH : (ec + 1) * QCH],
                            start=(ks == 0),
                            stop=(ks == 1),
                        )
                    nc.vector.tensor_copy(y_sb[:, ec * QCH : (ec + 1) * QCH], py[:])
                    if ec == C // QCH - 1:
                        nc.gpsimd.dma_start(
                            y.ap()[pb, tglob * P : (tglob + 1) * P, :], y_sb[:]
                        )

                ts_key = (pb, tglob)
                return emit

            def queue_p3(pb, pattnT, pqc):
                for ts in range(QCH // P):
                    for ec in range(C // QCH):
                        p3_queue.append(p3_block(pb, pattnT, pqc * (QCH // P) + ts, ec))

            def pop_p3(n):
                for _ in range(min(n, len(p3_queue))):
                    p3_queue.pop(0)()

            # ---- P1: projection chunk emitters --------------------------
            def make_p1(b):
                xb = xt.ap()[b].rearrange("(ko p) t -> p ko t", p=P)
                qkk_sb = projpool.tile([P, 3, T], fp16, tag="qkk", name="qkk")
                v2_sb = v2pool.tile([P, KT, HD + 1], fp16, tag="v2", name="v2")
                nc.gpsimd.memset(v2_sb[:, :, HD : HD + 1], 1.0)
                xchs = {}

                def load(tch):
                    tsl = slice(tch * TCH, (tch + 1) * TCH)
                    xch = xpool.tile([P, KT, TCH], fp16, tag="xch", name="xch")
                    nc.sync.dma_start(xch[:, 0 : KT // 2, :], xb[:, 0 : KT // 2, tsl])
                    nc.sync.dma_start(xch[:, KT // 2 :, :], xb[:, KT // 2 :, tsl])
                    xchs[tch] = xch

                def chunk(tch):
                    def emit():
                        if tch + 1 < NTCH and tch + 1 not in xchs:
                            load(tch + 1)
                        if b == 0 and tch == 4:
                            nc.sync.dma_start(wo_sb[:], wo3)
                        tsl = slice(tch * TCH, (tch + 1) * TCH)
                        xch = xchs.pop(tch)
                        for sub in range(3):
                            pp_full = psumY.tile([P, QCH], f32, tag="py", name="pp")
                            pp = pp_full[:, :TCH]
                            for k in range(KT):
                                nc.tensor.matmul(
                                    pp[:],
                                    wqk_sb[:, k, sub * P : (sub + 1) * P],
                                    xch[:, k, :],
                                    start=(k == 0),
                                    stop=(k == KT - 1),
                                )
                            nc.vector.tensor_tensor(
                                qkk_sb[:, sub, tsl],
                                pp[:],
                                bqk_sb[:, sub : sub + 1].to_broadcast((P, TCH)),
                                mybir.AluOpType.add,
                            )
                        for ts in range(TCH // P):
                            tidx = tch * (TCH // P) + ts
                            pv_full = psumY.tile([P, QCH], f32, tag="py", name="pv")
                            pv = pv_full[:, :HD]
                            for k in range(KT):
                                nc.tensor.matmul(
                                    pv[:],
                                    xch[:, k, ts * P : (ts + 1) * P],
                                    wv_sb[:, k, :],
                                    start=(k == 0),
                                    stop=(k == KT - 1),
                                )
                            nc.vector.tensor_tensor(
                                v2_sb[:, tidx, 0:HD],
                                pv[:],
                                bv_sb[:],
                                mybir.AluOpType.add,
                            )

                    return emit

                if b == 0:
                    xchs[0] = xch0
                    xchs[1] = xch1
                else:
                    load(0)
                return qkk_sb, v2_sb, [chunk(t) for t in range(NTCH)]

            # ---- P2: attention for one batch ----------------------------
            def emit_p2(b, qkk_sb, v2_sb):
                for qc in range(NQC):
                    q0 = qc * QCH
                    nfull = q0 // P
                    ntiles = nfull + QCH // P

                    for sub in range(2):
                        extiles = {}
                        if sub == 0:
                            attnT = apool.tile(
                                [P, 2, QCH], fp16, tag="attnT", name="attnT"
                            )
                        pava = psumAV.tile(
                            [P, 2, 2, HD + 1], f32, tag="pav", name="pava"
                        )
                        pavb = psumAV.tile(
                            [P, 2, 2, HD + 1], f32, tag="pav", name="pavb"
                        )
                        pavs = (pava, pavb)

                        def emit_scores(i):
                            if i < nfull:
                                nsl = slice(0, QCH)
                            else:
                                nsl = slice((i - nfull) * P, QCH)
                            ksl = slice(i * P, (i + 1) * P)
                            ex = expool.tile([P, 2, QCH], fp16, tag="ex", name="ex")
                            extiles[i] = ex
                            ps_s = psumS.tile([P, 2, QCH], f32, tag="ps", name="ps_s")
                            # concurrent pair: disjoint PE rows 0-63 / 64-127
                            nc.tensor.matmul(
                                ps_s[:, 0, nsl],
                                qkk_sb[0:64, 2, ksl],
                                qkk_sb[0:64, sub, q0 + nsl.start : q0 + QCH],
                                start=True,
                                stop=True,
                            )
                            nc.tensor.matmul(
                                ps_s[:, 1, nsl],
                                qkk_sb[64:128, 2, ksl],
                                qkk_sb[64:128, sub, q0 + nsl.start : q0 + QCH],
                                start=True,
                                stop=True,
                            )
                            nc.scalar.activation(
                                ex[:, :, nsl],
                                ps_s[:, :, nsl],
                                mybir.ActivationFunctionType.Exp,
                                scale=0.125,
                            )
                            if i >= nfull:
                                j = i - nfull
                                nc.vector.tensor_tensor(
                                    ex[:, :, j * P : (j + 1) * P],
                                    ex[:, :, j * P : (j + 1) * P],
                                    mask_sb[:, None, :].to_broadcast((P, 2, P)),
                                    mybir.AluOpType.mult,
                                )

                        def emit_av(i):
                            # AV natural orientation, diagonal-restricted; the
                            # 4 q-tile chains x 2 heads share 2 PSUM banks:
                            # pav[ts%2, half, 0:65], col 64 = denominator.
                            for ts in range(QCH // P):
                                if nfull + ts < i:
                                    continue
                                pav = pavs[ts // 2]
                                for half in range(2):
                                    # start only on the FIRST matmul into each
                                    # PSUM bank: start_tensor_calc marks the
                                    # whole 2KB zero region pending-zero, so
                                    # each chain's first write self-initializes
                                    nc.tensor.matmul(
                                        pav[:, ts % 2, half, :],
                                        extiles[i][:, half, ts * P : (ts + 1) * P],
                                        v2_sb[:, i, :],
                                        start=(i == 0 and ts % 2 == 0 and half == 0),
                                        stop=(i == nfull + ts),
                                        skip_group_check=True,
                                    )
                            # chain ts = i - nfull just stopped: normalize it
                            ts = i - nfull
                            if 0 <= ts < QCH // P:
                                pav = pavs[ts // 2]
                                rec = aspool.tile([P, 2], f32, tag="rec", name="rec")
                                nc.vector.reciprocal(rec[:], pav[:, ts % 2, :, HD])
                                attn = aspool.tile(
                                    [P, 2, HD], fp16, tag="attn", name="attn"
                                )
                                nc.vector.tensor_tensor(
                                    attn[:],
                                    pav[:, ts % 2, :, 0:HD],
                                    rec[:, :, None].to_broadcast((P, 2, HD)),
                                    mybir.AluOpType.mult,
                                )
                                # attn^T via PE transpose: [128 q, 128 ch] ->
                                # attnT[p, sub, ts*P + q] with ch = sub*128 + p
                                ptr = psumY.tile([P, P], fp16, tag="py", name="ptr")
                                nc.tensor.transpose(
                                    ptr[:],
                                    attn[:].rearrange("p a b -> p (a b)"),
                                    ident_sb[:],
                                )
                                nc.vector.tensor_copy(
                                    attnT[:, sub, ts * P : (ts + 1) * P], ptr[:]
                                )

                        # lag-1 pipeline over kv tiles; drain the deferred
                        # queues evenly across this sub's steps
                        p3_backlog = len(p3_queue)
                        p3_target = p3_backlog // 2 if sub == 0 else 0
                        for i in range(ntiles + 2):
                            if i < ntiles:
                                emit_scores(i)
                            if i >= 2:
                                emit_av(i - 2)
                            want = p3_target + (
                                (p3_backlog - p3_target) * (ntiles + 1 - i)
                            ) // (ntiles + 2)
                            pop_p3(len(p3_queue) - want)
                            # spread the next batch's projection chunks.
                            # During P2(b0): 6 of the 8 chunks, evenly.
                            # During P2(b1): chunk tch is due before its own
                            # q-chunk (tch//2), so the last two drain there.
                            if p1_queue and b == 0:
                                steps_done = (
                                    sum(2 * (qq * 4 + 6) for qq in range(qc))
                                    + sub * (ntiles + 2)
                                    + i
                                    + 1
                                )
                                if len(p1_done) < 6 and steps_done * 6 >= (
                                    len(p1_done) + 1
                                ) * 80:
                                    p1_queue.pop(0)()
                                    p1_done.append(1)
                            elif p1_queue and b == 1 and sub == 0 and i == 4:
                                if qc < 2:
                                    p1_queue.pop(0)()
                        if sub == 1:
                            if b == B - 1 and qc == NQC - 1:
                                for ts in range(QCH // P):
                                    for ec in range(C // QCH):
                                        p3_block(b, attnT, qc * (QCH // P) + ts, ec)()
                            else:
                                queue_p3(b, attnT, qc)

            # ---- whole-kernel emission ----------------------------------
            qkk0, v20, chunks0 = make_p1(0)
            for ch in chunks0:
                ch()
                pop_p3(2)
            qkk1, v21, chunks1 = make_p1(1)
            p1_queue.extend(chunks1)
            p1_done = []
            emit_p2(0, qkk0, v20)
            while p1_queue:
                p1_queue.pop(0)()
            emit_p2(1, qkk1, v21)
            pop_p3(len(p3_queue))

    nc.compile()
    return nc


def _prep_inputs(x, Wq, bq, Wk, bk, Wv, bv, Wo, bo):
    x = np.ascontiguousarray(np.asarray(x, dtype=np.float32))
    xt = np.ascontiguousarray(x.transpose(0, 2, 1)).astype(np.float16)
    Wq = np.asarray(Wq, dtype=np.float32)
    Wk = np.asarray(Wk, dtype=np.float32)
    Wv = np.asarray(Wv, dtype=np.float32)
    Wo = np.asarray(Wo, dtype=np.float32)
    bq = np.asarray(bq, dtype=np.float32)
    bk = np.asarray(bk, dtype=np.float32)
    bv = np.asarray(bv, dtype=np.float32)

    # mask[kj, qi] = 1 iff kj <= qi  (upper triangular incl. diag)
    mask = np.triu(np.ones((P, P), dtype=np.float16)).copy()
    ident = np.eye(P, dtype=np.float16)
    in_maps = []
    for c in range(NCORES):
        qs = slice(c * G * HD, (c + 1) * G * HD)
        ks = slice(c * HD, (c + 1) * HD)
        wqk_c = np.concatenate([Wq[:, qs], Wk[:, ks], Wk[:, ks]], axis=1)
        bq_c = bq[qs]
        bqk_c = np.stack(
            [bq_c[0:128], bq_c[128:256], np.concatenate([bk[ks], bk[ks]])], axis=1
        )
        in_maps.append(
            {
                "xt": xt,
                "wqk": np.ascontiguousarray(wqk_c).astype(np.float16),
                "wv": np.ascontiguousarray(Wv[:, ks]).astype(np.float16),
                "wo": np.ascontiguousarray(Wo[qs, :]).astype(np.float16),
                "bqk": np.ascontiguousarray(bqk_c),
                "bv": np.ascontiguousarray(bv[None, ks]),
                "mask": mask,
                "ident": ident,
            }
        )
    return in_maps


def kernel(x, Wq, bq, Wk, bk, Wv, bv, Wo, bo, _trace=False):
    # NTFF tracing is unavailable through this axon client; make sure a
    # stray BASS_TRACE=1 in the environment cannot divert the run path.
    if not _trace:
        os.environ["BASS_NEVER_TRACE"] = "1"
    if "nc" not in _CACHE:
        _CACHE["nc"] = _build()
    nc = _CACHE["nc"]
    in_maps = _prep_inputs(x, Wq, bq, Wk, bk, Wv, bv, Wo, bo)
    res = bass_utils.run_bass_kernel_spmd(
        nc, in_maps, core_ids=list(range(NCORES)), trace=_trace
    )
    bo = np.asarray(bo, dtype=np.float32)
    y = np.zeros((B, T, C), dtype=np.float32)
    for c in range(NCORES):
        y += res.results[c]["y"].astype(np.float32)
    y += bo
    if _trace:
        return y, res
    return y


# revision 22
# speedup vs baseline: 1.3008x; 1.0519x over previous
"""GQA forward kernel for Trainium2, 8-core tensor-parallel (group-aligned).

Problem: B=2, T=2048, D=2048, 32 Q heads / 8 KV heads, head_dim 64, causal.

Sharding: core c owns KV head c and its 4 Q heads (whole GQA group), both
batches.  Output projection is row-parallel Megatron style: each core
contracts its 256 attention-output channels against its slice of Wo and the
host sums the 8 partial outputs (+ bo).

All device dataflow is fp16 (fp32 PSUM accumulation), which halves HBM
traffic vs fp32 and runs matmuls at 1 row/cycle at any tile width.

Per-core dataflow:
  x^T [C, T] fp16 (host-transposed)
    -> QKK proj: lhsT = [Wq_p0 | Wq_p1 | Wk | Wk] -> Q^T [256, T], K^T dup [128, T]
    -> V proj: natural orientation -> V2 [T, 65] (V plus ones col for the
       softmax denominator), per 128-token tile
  attention per (batch, q-chunk of 512):
    S^T[kv, q] = matmul(lhsT=K^T tile [64,128], rhs=Q^T [64, nsl]); the two
      heads of a pair run on disjoint PE row groups (base partitions 0 / 64)
    expS = ACT Exp(S^T / 8) -> SBUF fp16  (no max-subtraction: |S/8| <= ~6)
    causal: column-sliced matmuls + one triangle mask-mult on diagonal tiles
    AV in NATURAL orientation (half the PE cost of the transposed form):
      pav[q-tile, head, 0:65] += matmul(lhsT=expS[kv, q-tile], rhs=V2[kv, 0:65])
      accumulated over kv tiles; col 64 is the denominator.
    normalize on DVE (reciprocal + mult) -> attn [q, 256] fp16
    attn^T via PE transpose -> attnT [ch, q] (lhsT layout for out-proj)
  out-proj: y[t, e] = matmul(lhsT=attnT [256, t], rhs=Wo_c [256, e]) -> fp16

Emission is software-pipelined by hand because every engine executes its
queue in order: scores(i+1) is emitted before AV(i), out-proj blocks of the
previous q-chunk and the NEXT batch's projection chunks are spread into the
ACT-bound attention steps so the PE never parks on the exp it is about to
consume.
"""

import os

import numpy as np

import concourse.mybir as mybir
import concourse.tile as tile
from concourse import bacc
from concourse import bass_utils

P = 128
B = 2
T = 2048
C = 2048
HD = 64
QH = 32
KVH = 8
G = QH // KVH  # 4
NCORES = 8
TCH = 256  # token chunk for projection phase
QCH = 512  # q chunk for attention phase
KT = C // P  # 16 contraction tiles
NTCH = T // TCH  # 8
NQC = T // QCH  # 4
f32 = mybir.dt.float32
fp16 = mybir.dt.float16

_CACHE = {}


def _build():
    nc = bacc.Bacc("TRN2", target_bir_lowering=False, debug=False, num_devices=NCORES)

    xt = nc.dram_tensor("xt", [B, C, T], fp16, kind="ExternalInput")
    wqk = nc.dram_tensor("wqk", [C, 384], fp16, kind="ExternalInput")
    wv = nc.dram_tensor("wv", [C, HD], fp16, kind="ExternalInput")
    wo = nc.dram_tensor("wo", [G * HD, C], fp16, kind="ExternalInput")
    bqk = nc.dram_tensor("bqk", [P, 3], f32, kind="ExternalInput")
    bv = nc.dram_tensor("bv", [1, HD], f32, kind="ExternalInput")
    maskd = nc.dram_tensor("mask", [P, P], fp16, kind="ExternalInput")
    identd = nc.dram_tensor("ident", [P, P], fp16, kind="ExternalInput")
    y = nc.dram_tensor("y", [B, T, C], fp16, kind="ExternalOutput")

    wqk3 = wqk.ap().rearrange("(ko p) m -> p ko m", p=P)
    wo3 = wo.ap().rearrange("(ko p) m -> p ko m", p=P)

    with tile.TileContext(nc) as tc:
        with (
            tc.tile_pool(name="const", bufs=1) as cpool,
            tc.tile_pool(name="x", bufs=3) as xpool,
            tc.tile_pool(name="proj", bufs=2) as projpool,
            tc.tile_pool(name="v2p", bufs=2) as v2pool,
            tc.tile_pool(name="kk", bufs=2) as kkpool,
            tc.tile_pool(name="exps", bufs=24) as expool,
            tc.tile_pool(name="attnT", bufs=3) as apool,
            tc.tile_pool(name="attn", bufs=4) as aspool,
            tc.tile_pool(name="y", bufs=4) as ypool,
            tc.tile_pool(name="psS", bufs=2, space="PSUM") as psumS,
            tc.tile_pool(name="psAV", bufs=2, space="PSUM") as psumAV,
            tc.tile_pool(name="psY", bufs=2, space="PSUM") as psumY,
        ):
            # ---- constants / weights (resident) ----
            # startup-critical DMA order: first half of wqk sub0 + x chunk 0
            # (the first 8 QKK matmuls need only these), then the rest
            wqk_sb = cpool.tile([P, KT, 384], fp16)
            xb0 = xt.ap()[0].rearrange("(ko p) t -> p ko t", p=P)
            xch0 = xpool.tile([P, KT, TCH], fp16, tag="xch", name="xch")
            nc.sync.dma_start(wqk_sb[:, 0 : KT // 2, 0:P], wqk3[:, 0 : KT // 2, 0:P])
            nc.sync.dma_start(xch0[:, 0 : KT // 2, :], xb0[:, 0 : KT // 2, 0:TCH])
            nc.sync.dma_start(wqk_sb[:, KT // 2 :, 0:P], wqk3[:, KT // 2 :, 0:P])
            nc.sync.dma_start(xch0[:, KT // 2 :, :], xb0[:, KT // 2 :, 0:TCH])
            xch1 = xpool.tile([P, KT, TCH], fp16, tag="xch", name="xch")
            nc.sync.dma_start(xch1[:, 0 : KT // 2, :], xb0[:, 0 : KT // 2, TCH : 2 * TCH])
            nc.sync.dma_start(xch1[:, KT // 2 :, :], xb0[:, KT // 2 :, TCH : 2 * TCH])
            for _s in range(1, 3):
                nc.sync.dma_start(
                    wqk_sb[:, :, _s * P : (_s + 1) * P], wqk3[:, :, _s * P : (_s + 1) * P]
                )
            bqk_sb = cpool.tile([P, 3], f32)
            nc.sync.dma_start(bqk_sb[:], bqk.ap())
            bv_sb = cpool.tile([P, HD], f32)
            nc.sync.dma_start(bv_sb[:], bv.ap().to_broadcast((P, HD)))
            mask_sb = cpool.tile([P, P], fp16)
            nc.sync.dma_start(mask_sb[:], maskd.ap())
            ident_sb = cpool.tile([P, P], fp16)
            nc.sync.dma_start(ident_sb[:], identd.ap())
            wo_sb = cpool.tile([P, 2, C], fp16)

            # ---- deferred-work queues (fill PE during ACT-bound stretches) --
            p3_queue = []  # out-proj (ts, ec) blocks of finished q-chunks
            p1_queue = []  # next batch's projection chunks
            p3_state = {}

            def p3_block(pb, pattnT, tglob, ec):
                # one y row-block [128, C] per q-tile: 4 (ec) matmul+copy
                # units sharing a y_sb row, one big DMA after the last ec
                def emit():
                    if ec == 0:
                        p3_state[ts_key] = ypool.tile(
                            [P, C], fp16, tag="ysb", name="ysb"
                        )
                    y_sb = p3_state[ts_key]
                    py = psumY.tile([P, QCH], f32, tag="py", name="py")
                    for ks in range(2):
                        nc.tensor.matmul(
                            py[:],
                            pattnT[:, ks, (tglob % 4) * P : (tglob % 4 + 1) * P],
                            wo_sb[:, ks, ec * QCH : (ec + 1) * QCH],
                            start=(ks == 0),
                            stop=(ks == 1),
                        )
                    nc.vector.tensor_copy(y_sb[:, ec * QCH : (ec + 1) * QCH], py[:])
                    if ec == C // QCH - 1:
                        nc.gpsimd.dma_start(
                            y.ap()[pb, tglob * P : (tglob + 1) * P, :], y_sb[:]
                        )

                ts_key = (pb, tglob)
                return emit

            def queue_p3(pb, pattnT, pqc):
                for ts in range(QCH // P):
                    for ec in range(C // QCH):
                        p3_queue.append(p3_block(pb, pattnT, pqc * (QCH // P) + ts, ec))

            def pop_p3(n):
                for _ in range(min(n, len(p3_queue))):
                    p3_queue.pop(0)()

            # ---- P1: projection chunk emitters --------------------------
            def make_p1(b):
                xb = xt.ap()[b].rearrange("(ko p) t -> p ko t", p=P)
                qkk_sb = projpool.tile([P, 3, T], fp16, tag="qkk", name="qkk")
                kk_sb = kkpool.tile([P, T], fp16, tag="kk", name="kk")
                v2_sb = v2pool.tile([P, KT, HD + 1], fp16, tag="v2", name="v2")
                nc.gpsimd.memset(v2_sb[:, :, HD : HD + 1], 1.0)
                xchs = {}

                def load(tch):
                    tsl = slice(tch * TCH, (tch + 1) * TCH)
                    xch = xpool.tile([P, KT, TCH], fp16, tag="xch", name="xch")
                    nc.sync.dma_start(xch[:, 0 : KT // 2, :], xb[:, 0 : KT // 2, tsl])
                    nc.sync.dma_start(xch[:, KT // 2 :, :], xb[:, KT // 2 :, tsl])
                    xchs[tch] = xch

                def chunk(tch):
                    def emit():
                        if tch + 1 < NTCH and tch + 1 not in xchs:
                            load(tch + 1)
                        if b == 0 and tch == 4:
                            nc.sync.dma_start(wo_sb[:], wo3)
                        tsl = slice(tch * TCH, (tch + 1) * TCH)
                        xch = xchs.pop(tch)
                        for sub in range(3):
                            pp_full = psumY.tile([P, QCH], f32, tag="py", name="pp")
                            pp = pp_full[:, :TCH]
                            for k in range(KT):
                                nc.tensor.matmul(
                                    pp[:],
                                    wqk_sb[:, k, sub * P : (sub + 1) * P],
                                    xch[:, k, :],
                                    start=(k == 0),
                                    stop=(k == KT - 1),
                                )
                            nc.vector.tensor_tensor(
                                qkk_sb[:, sub, tsl],
                                pp[:],
                                bqk_sb[:, sub : sub + 1].to_broadcast((P, TCH)),
                                mybir.AluOpType.add,
                            )
                        # K^T dup for the pair-1 score matmuls (rows 64:127)
                        nc.gpsimd.dma_start(
                            kk_sb[64:128, tsl], qkk_sb[0:64, 2, tsl]
                        )
                        # V natural from the V^T half of sub2 (PE transpose)
                        for ts in range(TCH // P):
                            tidx = tch * (TCH // P) + ts
                            psl = slice(tidx * P, (tidx + 1) * P)
                            pv2 = psumY.tile([P, QCH], fp16, tag="py", name="pv2")
                            nc.tensor.transpose(
                                pv2[:, 0:HD],
                                qkk_sb[64:128, 2, psl],
                                ident_sb[64:128, 64:128],
                            )
                            nc.vector.tensor_tensor(
                                v2_sb[:, tidx, 0:HD],
                                pv2[:, 0:HD],
                                bv_sb[:],
                                mybir.AluOpType.add,
                            )

                    return emit

                if b == 0:
                    xchs[0] = xch0
                    xchs[1] = xch1
                else:
                    load(0)
                return qkk_sb, kk_sb, v2_sb, [chunk(t) for t in range(NTCH)]

            # ---- P2: attention for one batch ----------------------------
            def emit_p2(b, qkk_sb, kk_sb, v2_sb):
                for qc in range(NQC):
                    q0 = qc * QCH
                    nfull = q0 // P
                    ntiles = nfull + QCH // P

                    for sub in range(2):
                        extiles = {}
                        if sub == 0:
                            attnT = apool.tile(
                                [P, 2, QCH], fp16, tag="attnT", name="attnT"
                            )
                        pava = psumAV.tile(
                            [P, 2, 2, HD + 1], f32, tag="pav", name="pava"
                        )
                        pavb = psumAV.tile(
                            [P, 2, 2, HD + 1], f32, tag="pav", name="pavb"
                        )
                        pavs = (pava, pavb)

                        def emit_scores(i):
                            if i < nfull:
                                nsl = slice(0, QCH)
                            else:
                                nsl = slice((i - nfull) * P, QCH)
                            ksl = slice(i * P, (i + 1) * P)
                            ex = expool.tile([P, 2, QCH], fp16, tag="ex", name="ex")
                            extiles[i] = ex
                            ps_s = psumS.tile([P, 2, QCH], f32, tag="ps", name="ps_s")
                            # concurrent pair: disjoint PE rows 0-63 / 64-127
                            nc.tensor.matmul(
                                ps_s[:, 0, nsl],
                                qkk_sb[0:64, 2, ksl],
                                qkk_sb[0:64, sub, q0 + nsl.start : q0 + QCH],
                                start=True,
                                stop=True,
                            )
                            nc.tensor.matmul(
                                ps_s[:, 1, nsl],
                                kk_sb[64:128, ksl],
                                qkk_sb[64:128, sub, q0 + nsl.start : q0 + QCH],
                                start=True,
                                stop=True,
                            )
                            nc.scalar.activation(
                                ex[:, :, nsl],
                                ps_s[:, :, nsl],
                                mybir.ActivationFunctionType.Exp,
                                scale=0.125,
                            )
                            if i >= nfull:
                                j = i - nfull
                                nc.vector.tensor_tensor(
                                    ex[:, :, j * P : (j + 1) * P],
                                    ex[:, :, j * P : (j + 1) * P],
                                    mask_sb[:, None, :].to_broadcast((P, 2, P)),
                                    mybir.AluOpType.mult,
                                )

                        def emit_av(i):
                            # AV natural orientation, diagonal-restricted; the
                            # 4 q-tile chains x 2 heads share 2 PSUM banks:
                            # pav[ts%2, half, 0:65], col 64 = denominator.
                            for ts in range(QCH // P):
                                if nfull + ts < i:
                                    continue
                                pav = pavs[ts // 2]
                                for half in range(2):
                                    # start only on the FIRST matmul into each
                                    # PSUM bank: start_tensor_calc marks the
                                    # whole 2KB zero region pending-zero, so
                                    # each chain's first write self-initializes
                                    nc.tensor.matmul(
                                        pav[:, ts % 2, half, :],
                                        extiles[i][:, half, ts * P : (ts + 1) * P],
                                        v2_sb[:, i, :],
                                        start=(i == 0 and ts % 2 == 0 and half == 0),
                                        stop=(i == nfull + ts),
                                        skip_group_check=True,
                                    )
                            # chain ts = i - nfull just stopped: normalize it
                            ts = i - nfull
                            if 0 <= ts < QCH // P:
                                pav = pavs[ts // 2]
                                rec = aspool.tile([P, 2], f32, tag="rec", name="rec")
                                nc.vector.reciprocal(rec[:], pav[:, ts % 2, :, HD])
                                attn = aspool.tile(
                                    [P, 2, HD], fp16, tag="attn", name="attn"
                                )
                                nc.vector.tensor_tensor(
                                    attn[:],
                                    pav[:, ts % 2, :, 0:HD],
                                    rec[:, :, None].to_broadcast((P, 2, HD)),
                                    mybir.AluOpType.mult,
                                )
                                # attn^T via PE transpose: [128 q, 128 ch] ->
                                # attnT[p, sub, ts*P + q] with ch = sub*128 + p
                                ptr = psumY.tile([P, P], fp16, tag="py", name="ptr")
                                nc.tensor.transpose(
                                    ptr[:],
                                    attn[:].rearrange("p a b -> p (a b)"),
                                    ident_sb[:],
                                )
                                nc.vector.tensor_copy(
                                    attnT[:, sub, ts * P : (ts + 1) * P], ptr[:]
                                )

                        # lag-1 pipeline over kv tiles; drain the deferred
                        # queues evenly across this sub's steps
                        p3_backlog = len(p3_queue)
                        p3_target = p3_backlog // 2 if sub == 0 else 0
                        for i in range(ntiles + 2):
                            if i < ntiles:
                                emit_scores(i)
                            if i >= 2:
                                emit_av(i - 2)
                            want = p3_target + (
                                (p3_backlog - p3_target) * (ntiles + 1 - i)
                            ) // (ntiles + 2)
                            pop_p3(len(p3_queue) - want)
                            # spread the next batch's projection chunks.
                            # During P2(b0): 6 of the 8 chunks, evenly.
                            # During P2(b1): chunk tch is due before its own
                            # q-chunk (tch//2), so the last two drain there.
                            if p1_queue and b == 0:
                                steps_done = (
                                    sum(2 * (qq * 4 + 6) for qq in range(qc))
                                    + sub * (ntiles + 2)
                                    + i
                                    + 1
                                )
                                if len(p1_done) < 6 and steps_done * 6 >= (
                                    len(p1_done) + 1
                                ) * 80:
                                    p1_queue.pop(0)()
                                    p1_done.append(1)
                            elif p1_queue and b == 1 and sub == 0 and i == 4:
                                if qc < 2:
                                    p1_queue.pop(0)()
                        if sub == 1:
                            if b == B - 1 and qc == NQC - 1:
                                for ts in range(QCH // P):
                                    for ec in range(C // QCH):
                                        p3_block(b, attnT, qc * (QCH // P) + ts, ec)()
                            else:
                                queue_p3(b, attnT, qc)

            # ---- whole-kernel emission ----------------------------------
            qkk0, kk0, v20, chunks0 = make_p1(0)
            for ch in chunks0:
                ch()
                pop_p3(2)
            qkk1, kk1, v21, chunks1 = make_p1(1)
            p1_queue.extend(chunks1)
            p1_done = []
            emit_p2(0, qkk0, kk0, v20)
            while p1_queue:
                p1_queue.pop(0)()
            emit_p2(1, qkk1, kk1, v21)
            pop_p3(len(p3_queue))

    nc.compile()
    return nc


def _prep_inputs(x, Wq, bq, Wk, bk, Wv, bv, Wo, bo):
    x = np.ascontiguousarray(np.asarray(x, dtype=np.float32))
    xt = np.ascontiguousarray(x.transpose(0, 2, 1)).astype(np.float16)
    Wq = np.asarray(Wq, dtype=np.float32)
    Wk = np.asarray(Wk, dtype=np.float32)
    Wv = np.asarray(Wv, dtype=np.float32)
    Wo = np.asarray(Wo, dtype=np.float32)
    bq = np.asarray(bq, dtype=np.float32)
    bk = np.asarray(bk, dtype=np.float32)
    bv = np.asarray(bv, dtype=np.float32)

    # mask[kj, qi] = 1 iff kj <= qi  (upper triangular incl. diag)
    mask = np.triu(np.ones((P, P), dtype=np.float16)).copy()
    ident = np.eye(P, dtype=np.float16)
    in_maps = []
    for c in range(NCORES):
        qs = slice(c * G * HD, (c + 1) * G * HD)
        ks = slice(c * HD, (c + 1) * HD)
        wqk_c = np.concatenate([Wq[:, qs], Wk[:, ks], Wv[:, ks]], axis=1)
        bq_c = bq[qs]
        bqk_c = np.stack(
            [bq_c[0:128], bq_c[128:256], np.concatenate([bk[ks], 0 * bv[ks]])], axis=1
        )
        in_maps.append(
            {
                "xt": xt,
                "wqk": np.ascontiguousarray(wqk_c).astype(np.float16),
                "wv": np.ascontiguousarray(Wv[:, ks]).astype(np.float16),
                "wo": np.ascontiguousarray(Wo[qs, :]).astype(np.float16),
                "bqk": np.ascontiguousarray(bqk_c),
                "bv": np.ascontiguousarray(bv[None, ks]),
                "mask": mask,
                "ident": ident,
            }
        )
    return in_maps


def kernel(x, Wq, bq, Wk, bk, Wv, bv, Wo, bo, _trace=False):
    # NTFF tracing is unavailable through this axon client; make sure a
    # stray BASS_TRACE=1 in the environment cannot divert the run path.
    if not _trace:
        os.environ["BASS_NEVER_TRACE"] = "1"
    if "nc" not in _CACHE:
        _CACHE["nc"] = _build()
    nc = _CACHE["nc"]
    in_maps = _prep_inputs(x, Wq, bq, Wk, bk, Wv, bv, Wo, bo)
    res = bass_utils.run_bass_kernel_spmd(
        nc, in_maps, core_ids=list(range(NCORES)), trace=_trace
    )
    bo = np.asarray(bo, dtype=np.float32)
    y = np.zeros((B, T, C), dtype=np.float32)
    for c in range(NCORES):
        y += res.results[c]["y"].astype(np.float32)
    y += bo
    if _trace:
        return y, res
    return y


# revision 65
# speedup vs baseline: 1.3994x; 1.0758x over previous
"""GQA forward kernel for Trainium2, 8-core tensor-parallel (group-aligned).

Problem: B=2, T=2048, D=2048, 32 Q heads / 8 KV heads, head_dim 64, causal.

Sharding: core c owns KV head c and its 4 Q heads (whole GQA group), both
batches.  Output projection is row-parallel Megatron style: each core
contracts its 256 attention-output channels against its slice of Wo and the
host sums the 8 partial outputs (+ bo).

All device dataflow is fp16 (fp32 PSUM accumulation), which halves HBM
traffic vs fp32 and runs matmuls at 1 row/cycle at any tile width.

Per-core dataflow:
  x^T [C, T] fp16 (host-transposed)
    -> QKK proj: lhsT = [Wq_p0 | Wq_p1 | Wk | Wk] -> Q^T [256, T], K^T dup [128, T]
    -> V proj: natural orientation -> V2 [T, 65] (V plus ones col for the
       softmax denominator), per 128-token tile
  attention per (batch, q-chunk of 512):
    S^T[kv, q] = matmul(lhsT=K^T tile [64,128], rhs=Q^T [64, nsl]); the two
      heads of a pair run on disjoint PE row groups (base partitions 0 / 64)
    expS = ACT Exp(S^T / 8) -> SBUF fp16  (no max-subtraction: |S/8| <= ~6)
    causal: column-sliced matmuls + one triangle mask-mult on diagonal tiles
    AV in NATURAL orientation (half the PE cost of the transposed form):
      pav[q-tile, head, 0:65] += matmul(lhsT=expS[kv, q-tile], rhs=V2[kv, 0:65])
      accumulated over kv tiles; col 64 is the denominator.
    normalize on DVE (reciprocal + mult) -> attn [q, 256] fp16
    attn^T via PE transpose -> attnT [ch, q] (lhsT layout for out-proj)
  out-proj: y[t, e] = matmul(lhsT=attnT [256, t], rhs=Wo_c [256, e]) -> fp16

Emission is software-pipelined by hand because every engine executes its
queue in order: scores(i+1) is emitted before AV(i), out-proj blocks of the
previous q-chunk and the NEXT batch's projection chunks are spread into the
ACT-bound attention steps so the PE never parks on the exp it is about to
consume.
"""

import os

import numpy as np

import concourse.mybir as mybir
import concourse.tile as tile
from concourse import bacc
from concourse import bass_utils

P = 128
B = 2
T = 2048
C = 2048
HD = 64
QH = 32
KVH = 8
G = QH // KVH  # 4
NCORES = 8
TCH = 256  # token chunk for projection phase
QCH = 512  # q chunk for attention phase
KT = C // P  # 16 contraction tiles
NTCH = T // TCH  # 8
NQC = T // QCH  # 4
f32 = mybir.dt.float32
fp16 = mybir.dt.float16

_CACHE = {}


def _build():
    nc = bacc.Bacc("TRN2", target_bir_lowering=False, debug=False, num_devices=NCORES)

    xt = nc.dram_tensor("xt", [B, C, T], fp16, kind="ExternalInput")
    wqk = nc.dram_tensor("wqk", [C, 384], fp16, kind="ExternalInput")
    wv = nc.dram_tensor("wv", [C, HD], fp16, kind="ExternalInput")
    wo = nc.dram_tensor("wo", [G * HD, C], fp16, kind="ExternalInput")
    bqk = nc.dram_tensor("bqk", [P, 3], f32, kind="ExternalInput")
    bv = nc.dram_tensor("bv", [1, HD], f32, kind="ExternalInput")
    maskd = nc.dram_tensor("mask", [P, P], fp16, kind="ExternalInput")
    identd = nc.dram_tensor("ident", [P, P], fp16, kind="ExternalInput")
    y = nc.dram_tensor("y", [B, T, C], fp16, kind="ExternalOutput")

    wqk3 = wqk.ap().rearrange("(ko p) m -> p ko m", p=P)
    wo3 = wo.ap().rearrange("(ko p) m -> p ko m", p=P)

    with tile.TileContext(nc) as tc:
        with (
            tc.tile_pool(name="const", bufs=1) as cpool,
            tc.tile_pool(name="x", bufs=3) as xpool,
            tc.tile_pool(name="proj", bufs=2) as projpool,
            tc.tile_pool(name="v2p", bufs=2) as v2pool,
            tc.tile_pool(name="kk", bufs=2) as kkpool,
            tc.tile_pool(name="exps", bufs=32) as expool,
            tc.tile_pool(name="attnT", bufs=4) as apool,
            tc.tile_pool(name="attn", bufs=4) as aspool,
            tc.tile_pool(name="y", bufs=4) as ypool,
            tc.tile_pool(name="psS", bufs=2, space="PSUM") as psumS,
            tc.tile_pool(name="psAV", bufs=2, space="PSUM") as psumAV,
            tc.tile_pool(name="psY", bufs=2, space="PSUM") as psumY,
        ):
            # ---- constants / weights (resident) ----
            # startup-critical DMA order: first half of wqk sub0 + x chunk 0
            # (the first 8 QKK matmuls need only these), then the rest
            # cols 0:256 (both Q subs together) have 512B contiguous rows ->
            # full DMA rate; quarter-granularity first tiles cut the latency
            # to the very first matmul
            wqk_sb = cpool.tile([P, KT, 384], fp16)
            xb0 = xt.ap()[0].rearrange("(ko p) t -> p ko t", p=P)
            xch0 = xpool.tile([P, KT, TCH], fp16, tag="xch", name="xch")
            nc.sync.dma_start(wqk_sb[:, 0:4, :], wqk3[:, 0:4, :])
            nc.sync.dma_start(xch0[:, 0:4, :], xb0[:, 0:4, 0:TCH])
            nc.sync.dma_start(wqk_sb[:, 4:8, :], wqk3[:, 4:8, :])
            nc.sync.dma_start(xch0[:, 4:8, :], xb0[:, 4:8, 0:TCH])
            nc.sync.dma_start(wqk_sb[:, 8:KT, :], wqk3[:, 8:KT, :])
            nc.sync.dma_start(xch0[:, 8:KT, :], xb0[:, 8:KT, 0:TCH])
            xch1 = xpool.tile([P, KT, TCH], fp16, tag="xch", name="xch")
            nc.sync.dma_start(xch1[:, 0 : KT // 2, :], xb0[:, 0 : KT // 2, TCH : 2 * TCH])
            nc.sync.dma_start(xch1[:, KT // 2 :, :], xb0[:, KT // 2 :, TCH : 2 * TCH])
            bqk_sb = cpool.tile([P, 3], f32)
            nc.sync.dma_start(bqk_sb[:], bqk.ap())
            bv_sb = cpool.tile([P, HD], f32)
            nc.sync.dma_start(bv_sb[:], bv.ap().to_broadcast((P, HD)))
            mask_sb = cpool.tile([P, P], fp16)
            nc.sync.dma_start(mask_sb[:], maskd.ap())
            ident_sb = cpool.tile([P, P], fp16)
            nc.sync.dma_start(ident_sb[:], identd.ap())
            wo_sb = cpool.tile([P, 2, C], fp16)
            # PE warmup scratch: dummy matmuls keep the PE pstate ramped
            # through startup DMA waits (results are never read)
            scratch_sb = cpool.tile([P, QCH], fp16)
            nc.gpsimd.memset(scratch_sb[:], 0.0)

            def emit_dummy(n):
                for _ in range(n):
                    dmy = psumY.tile([P, QCH], f32, tag="py", name="dmy")
                    nc.tensor.matmul(
                        dmy[:],
                        scratch_sb[:, 0:P],
                        scratch_sb[:],
                        start=True,
                        stop=True,
                    )

            # ---- deferred-work queues (fill PE during ACT-bound stretches) --
            p3_queue = []  # out-proj (ts, ec) blocks of finished q-chunks
            p1_queue = []  # next batch's projection chunks
            p3_state = {}
            in_p1_phase = [True]  # psAV banks are free during projections

            def p3_block(pb, pattnT, tglob, ec):
                # one y row-block [128, C] per q-tile: 4 (ec) matmul+copy
                # units sharing a y_sb row, one big DMA after the last ec
                def emit():
                    if ec == 0:
                        p3_state[ts_key] = ypool.tile(
                            [P, C], fp16, tag="ysb", name="ysb"
                        )
                    y_sb = p3_state[ts_key]
                    if in_p1_phase[0] and (ec + tglob) % 2 == 0:
                        py = psumAV.tile([P, QCH], f32, tag="pav", name="py2")
                    else:
                        py = psumY.tile([P, QCH], f32, tag="py", name="py")
                    for ks in range(2):
                        nc.tensor.matmul(
                            py[:],
                            pattnT[:, ks, (tglob % 4) * P : (tglob % 4 + 1) * P],
                            wo_sb[:, ks, ec * QCH : (ec + 1) * QCH],
                            start=(ks == 0),
                            stop=(ks == 1),
                        )
                    last = pb == B - 1 and tglob >= 12
                    if last and ec % 2 == 1:
                        nc.scalar.copy(y_sb[:, ec * QCH : (ec + 1) * QCH], py[:])
                    else:
                        nc.vector.tensor_copy(y_sb[:, ec * QCH : (ec + 1) * QCH], py[:])
                    if last and tglob >= 14:
                        # tail latency: half-row DMAs via the fast HWDGE path,
                        # issued as soon as each half is copied
                        if ec == 1:
                            nc.sync.dma_start(
                                y.ap()[pb, tglob * P : (tglob + 1) * P, 0 : C // 2],
                                y_sb[:, 0 : C // 2],
                            )
                        elif ec == 3:
                            nc.sync.dma_start(
                                y.ap()[pb, tglob * P : (tglob + 1) * P, C // 2 :],
                                y_sb[:, C // 2 :],
                            )
                    elif ec == C // QCH - 1:
                        nc.gpsimd.dma_start(
                            y.ap()[pb, tglob * P : (tglob + 1) * P, :], y_sb[:]
                        )

                ts_key = (pb, tglob)
                return emit

            def queue_p3(pb, pattnT, pqc):
                for ts in range(QCH // P):
                    for ec in range(C // QCH):
                        p3_queue.append(p3_block(pb, pattnT, pqc * (QCH // P) + ts, ec))

            def pop_p3(n):
                for _ in range(min(n, len(p3_queue))):
                    p3_queue.pop(0)()

            # ---- P1: projection chunk emitters --------------------------
            def make_p1(b):
                xb = xt.ap()[b].rearrange("(ko p) t -> p ko t", p=P)
                qkk_sb = projpool.tile([P, 3, T], fp16, tag="qkk", name="qkk")
                kk_sb = kkpool.tile([P, T], fp16, tag="kk", name="kk")
                v2_sb = v2pool.tile([P, KT, HD + 1], fp16, tag="v2", name="v2")
                nc.gpsimd.memset(v2_sb[:, :, HD : HD + 1], 1.0)
                xchs = {}

                def load(tch):
                    tsl = slice(tch * TCH, (tch + 1) * TCH)
                    xch = xpool.tile([P, KT, TCH], fp16, tag="xch", name="xch")
                    nc.sync.dma_start(xch[:, 0 : KT // 2, :], xb[:, 0 : KT // 2, tsl])
                    nc.sync.dma_start(xch[:, KT // 2 :, :], xb[:, KT // 2 :, tsl])
                    xchs[tch] = xch

                def sub_proj(tch, sub):
                    # one ~1us unit: a single sub-projection chain
                    def emit():
                        if sub == 0:
                            if tch + 1 < NTCH and tch + 1 not in xchs:
                                load(tch + 1)
                            if b == 0 and tch == 4:
                                nc.sync.dma_start(wo_sb[:], wo3)
                        tsl = slice(tch * TCH, (tch + 1) * TCH)
                        xch = xchs[tch]
                        pp_full = psumY.tile([P, QCH], f32, tag="py", name="pp")
                        pp = pp_full[:, :TCH]
                        for k in range(KT):
                            nc.tensor.matmul(
                                pp[:],
                                wqk_sb[:, k, sub * P : (sub + 1) * P],
                                xch[:, k, :],
                                start=(k == 0),
                                stop=(k == KT - 1),
                            )
                        nc.vector.tensor_tensor(
                            qkk_sb[:, sub, tsl],
                            pp[:],
                            bqk_sb[:, sub : sub + 1].to_broadcast((P, TCH)),
                            mybir.AluOpType.add,
                        )

                    return emit

                def kv_finish(tch):
                    # K^T dup + V-natural transposes for one chunk
                    def emit():
                        tsl = slice(tch * TCH, (tch + 1) * TCH)
                        xchs.pop(tch, None)
                        nc.gpsimd.dma_start(
                            kk_sb[64:128, tsl], qkk_sb[0:64, 2, tsl]
                        )
                        for ts in range(TCH // P):
                            tidx = tch * (TCH // P) + ts
                            psl = slice(tidx * P, (tidx + 1) * P)
                            pv2 = psumY.tile([P, QCH], fp16, tag="py", name="pv2")
                            nc.tensor.transpose(
                                pv2[:, 0:HD],
                                qkk_sb[64:128, 2, psl],
                                ident_sb[64:128, 64:128],
                            )
                            nc.vector.tensor_tensor(
                                v2_sb[:, tidx, 0:HD],
                                pv2[:, 0:HD],
                                bv_sb[:],
                                mybir.AluOpType.add,
                            )

                    return emit

                if b == 0:
                    xchs[0] = xch0
                    xchs[1] = xch1
                else:
                    load(0)
                units = []
                for t in range(NTCH):
                    grp = [sub_proj(t, 0)]
                    if t > 0:
                        grp.append(kv_finish(t - 1))
                    grp += [sub_proj(t, 1), sub_proj(t, 2)]
                    units.append(grp)
                units.append([kv_finish(NTCH - 1)])
                return qkk_sb, kk_sb, v2_sb, units

            # ---- P2: attention for one batch ----------------------------
            def emit_p2(b, qkk_sb, kk_sb, v2_sb):
                for qc in range(NQC):
                    q0 = qc * QCH
                    nfull = q0 // P
                    ntiles = nfull + QCH // P

                    for sub in range(2):
                        extiles = {}
                        if sub == 0:
                            attnT = apool.tile(
                                [P, 2, QCH], fp16, tag="attnT", name="attnT"
                            )
                        pava = psumAV.tile(
                            [P, 2, 2, HD + 1], f32, tag="pav", name="pava"
                        )
                        pavb = psumAV.tile(
                            [P, 2, 2, HD + 1], f32, tag="pav", name="pavb"
                        )
                        pavs = (pava, pavb)

                        def emit_scores(i):
                            if i < nfull:
                                nsl = slice(0, QCH)
                            else:
                                nsl = slice((i - nfull) * P, QCH)
                            ksl = slice(i * P, (i + 1) * P)
                            ex = expool.tile([P, 2, QCH], fp16, tag="ex", name="ex")
                            extiles[i] = ex
                            ps_s = psumS.tile([P, 2, QCH], f32, tag="ps", name="ps_s")
                            # concurrent pair: disjoint PE rows 0-63 / 64-127
                            nc.tensor.matmul(
                                ps_s[:, 0, nsl],
                                qkk_sb[0:64, 2, ksl],
                                qkk_sb[0:64, sub, q0 + nsl.start : q0 + QCH],
                                start=True,
                                stop=True,
                            )
                            nc.tensor.matmul(
                                ps_s[:, 1, nsl],
                                kk_sb[64:128, ksl],
                                qkk_sb[64:128, sub, q0 + nsl.start : q0 + QCH],
                                start=True,
                                stop=True,
                            )
                            nc.scalar.activation(
                                ex[:, :, nsl],
                                ps_s[:, :, nsl],
                                mybir.ActivationFunctionType.Exp,
                                scale=0.125,
                            )
                            if i >= nfull:
                                j = i - nfull
                                nc.vector.tensor_tensor(
                                    ex[:, :, j * P : (j + 1) * P],
                                    ex[:, :, j * P : (j + 1) * P],
                                    mask_sb[:, None, :].to_broadcast((P, 2, P)),
                                    mybir.AluOpType.mult,
                                )

                        def emit_av(i):
                            # AV natural orientation, diagonal-restricted; the
                            # 4 q-tile chains x 2 heads share 2 PSUM banks:
                            # pav[ts%2, half, 0:65], col 64 = denominator.
                            for ts in range(QCH // P):
                                if nfull + ts < i:
                                    continue
                                pav = pavs[ts // 2]
                                for half in range(2):
                                    # start only on the FIRST matmul into each
                                    # PSUM bank: start_tensor_calc marks the
                                    # whole 2KB zero region pending-zero, so
                                    # each chain's first write self-initializes
                                    nc.tensor.matmul(
                                        pav[:, ts % 2, half, :],
                                        extiles[i][:, half, ts * P : (ts + 1) * P],
                                        v2_sb[:, i, :],
                                        start=(i == 0 and ts % 2 == 0 and half == 0),
                                        stop=(i == nfull + ts),
                                        skip_group_check=True,
                                    )
                            # chain ts = i - nfull just stopped: normalize it
                            ts = i - nfull
                            if 0 <= ts < QCH // P:
                                pav = pavs[ts // 2]
                                rec = aspool.tile([P, 2], f32, tag="rec", name="rec")
                                nc.vector.reciprocal(rec[:], pav[:, ts % 2, :, HD])
                                attn = aspool.tile(
                                    [P, 2, HD], fp16, tag="attn", name="attn"
                                )
                                nc.vector.tensor_tensor(
                                    attn[:],
                                    pav[:, ts % 2, :, 0:HD],
                                    rec[:, :, None].to_broadcast((P, 2, HD)),
                                    mybir.AluOpType.mult,
                                )
                                # attn^T via PE transpose: [128 q, 128 ch] ->
                                # attnT[p, sub, ts*P + q] with ch = sub*128 + p
                                ptr = psumY.tile([P, P], fp16, tag="py", name="ptr")
                                nc.tensor.transpose(
                                    ptr[:],
                                    attn[:].rearrange("p a b -> p (a b)"),
                                    ident_sb[:],
                                )
                                nc.vector.tensor_copy(
                                    attnT[:, sub, ts * P : (ts + 1) * P], ptr[:]
                                )
                                if b == B - 1 and qc == NQC - 1 and sub == 1:
                                    in_p1_phase[0] = True
                                    for ec in range(C // QCH):
                                        p3_block(
                                            b, attnT, qc * (QCH // P) + ts, ec
                                        )()
                                    in_p1_phase[0] = False

                        # lag-1 pipeline over kv tiles; drain the deferred
                        # queues evenly across this sub's steps
                        if sub == 0:
                            qc_popped[0] = 0
                        p3_backlog = len(p3_queue)
                        p3_target = p3_backlog // 2 if sub == 0 else 0
                        # in the last batch, hold back out-proj filler for the
                        # late (ACT-heavy) q-chunks
                        if b == B - 1:
                            keep = (8, 6, 0, 0)[qc]
                            p3_target = max(p3_target, min(keep, p3_backlog))
                        for i in range(ntiles + 2):
                            if i < ntiles:
                                emit_scores(i)
                            if i >= 2:
                                emit_av(i - 2)
                            want = p3_target + (
                                (p3_backlog - p3_target) * (ntiles + 1 - i)
                            ) // (ntiles + 2)
                            want = max(want, p3_target)
                            pop_p3(len(p3_queue) - want)
                            # spread the next batch's projection chunks.
                            # During P2(b0): 6 of the 8 chunks, evenly.
                            # During P2(b1): chunk tch is due before its own
                            # q-chunk (tch//2), so the last two drain there.
                            if p1_queue and b == 0:
                                quota = (4, 4, 8, 8)[qc]
                                k = sub * (ntiles + 2) + i + 1
                                steps_qc = 2 * (ntiles + 2)
                                if qc_popped[0] < quota and k * (quota + 1) >= (
                                    qc_popped[0] + 1
                                ) * steps_qc:
                                    p1_queue.pop(0)()
                                    qc_popped[0] += 1
                            elif p1_queue and b == 1 and sub == 0 and (
                                (qc < 2 and i in (2, 5, 8)) or (qc == 2 and i in (2, 5))
                            ):
                                p1_queue.pop(0)()
                        if sub == 1 and not (b == B - 1 and qc == NQC - 1):
                            queue_p3(b, attnT, qc)

            # ---- whole-kernel emission ----------------------------------
            qkk0, kk0, v20, chunks0 = make_p1(0)
            emit_dummy(4)
            for ci, ch in enumerate(chunks0):
                for u in ch:
                    u()
                pop_p3(2)
                if ci < 2:
                    emit_dummy(0)
            qkk1, kk1, v21, chunks1 = make_p1(1)
            for ch in chunks1:
                p1_queue.extend(ch)
            p1_done = []
            qc_popped = [0]
            in_p1_phase[0] = False
            emit_p2(0, qkk0, kk0, v20)
            in_p1_phase[0] = True
            while p1_queue:
                p1_queue.pop(0)()
            in_p1_phase[0] = False
            emit_p2(1, qkk1, kk1, v21)
            pop_p3(len(p3_queue))

    nc.compile()
    return nc


def _prep_inputs(x, Wq, bq, Wk, bk, Wv, bv, Wo, bo):
    x = np.ascontiguousarray(np.asarray(x, dtype=np.float32))
    xt = np.ascontiguousarray(x.transpose(0, 2, 1)).astype(np.float16)
    Wq = np.asarray(Wq, dtype=np.float32)
    Wk = np.asarray(Wk, dtype=np.float32)
    Wv = np.asarray(Wv, dtype=np.float32)
    Wo = np.asarray(Wo, dtype=np.float32)
    bq = np.asarray(bq, dtype=np.float32)
    bk = np.asarray(bk, dtype=np.float32)
    bv = np.asarray(bv, dtype=np.float32)

    # mask[kj, qi] = 1 iff kj <= qi  (upper triangular incl. diag)
    mask = np.triu(np.ones((P, P), dtype=np.float16)).copy()
    ident = np.eye(P, dtype=np.float16)
    in_maps = []
    for c in range(NCORES):
        qs = slice(c * G * HD, (c + 1) * G * HD)
        ks = slice(c * HD, (c + 1) * HD)
        wqk_c = np.concatenate([Wq[:, qs], Wk[:, ks], Wv[:, ks]], axis=1)
        bq_c = bq[qs]
        bqk_c = np.stack(
            [bq_c[0:128], bq_c[128:256], np.concatenate([bk[ks], 0 * bv[ks]])], axis=1
        )
        in_maps.append(
            {
                "xt": xt,
                "wqk": np.ascontiguousarray(wqk_c).astype(np.float16),
                "wv": np.ascontiguousarray(Wv[:, ks]).astype(np.float16),
                "wo": np.ascontiguousarray(Wo[qs, :]).astype(np.float16),
                "bqk": np.ascontiguousarray(bqk_c),
                "bv": np.ascontiguousarray(bv[None, ks]),
                "mask": mask,
                "ident": ident,
            }
        )
    return in_maps


def kernel(x, Wq, bq, Wk, bk, Wv, bv, Wo, bo, _trace=False):
    # NTFF tracing is unavailable through this axon client; make sure a
    # stray BASS_TRACE=1 in the environment cannot divert the run path.
    if not _trace:
        os.environ["BASS_NEVER_TRACE"] = "1"
    if "nc" not in _CACHE:
        _CACHE["nc"] = _build()
    nc = _CACHE["nc"]
    in_maps = _prep_inputs(x, Wq, bq, Wk, bk, Wv, bv, Wo, bo)
    res = bass_utils.run_bass_kernel_spmd(
        nc, in_maps, core_ids=list(range(NCORES)), trace=_trace
    )
    bo = np.asarray(bo, dtype=np.float32)
    y = np.zeros((B, T, C), dtype=np.float32)
    for c in range(NCORES):
        y += res.results[c]["y"].astype(np.float32)
    y += bo
    if _trace:
        return y, res
    return y


# revision 69
# speedup vs baseline: 1.4004x; 1.0007x over previous
"""GQA forward kernel for Trainium2, 8-core tensor-parallel (group-aligned).

Problem: B=2, T=2048, D=2048, 32 Q heads / 8 KV heads, head_dim 64, causal.

Sharding: core c owns KV head c and its 4 Q heads (whole GQA group), both
batches.  Output projection is row-parallel Megatron style: each core
contracts its 256 attention-output channels against its slice of Wo and the
host sums the 8 partial outputs (+ bo).

All device dataflow is fp16 (fp32 PSUM accumulation), which halves HBM
traffic vs fp32 and runs matmuls at 1 row/cycle at any tile width.

Per-core dataflow:
  x^T [C, T] fp16 (host-transposed)
    -> QKK proj: lhsT = [Wq_p0 | Wq_p1 | Wk | Wk] -> Q^T [256, T], K^T dup [128, T]
    -> V proj: natural orientation -> V2 [T, 65] (V plus ones col for the
       softmax denominator), per 128-token tile
  attention per (batch, q-chunk of 512):
    S^T[kv, q] = matmul(lhsT=K^T tile [64,128], rhs=Q^T [64, nsl]); the two
      heads of a pair run on disjoint PE row groups (base partitions 0 / 64)
    expS = ACT Exp(S^T / 8) -> SBUF fp16  (no max-subtraction: |S/8| <= ~6)
    causal: column-sliced matmuls + one triangle mask-mult on diagonal tiles
    AV in NATURAL orientation (half the PE cost of the transposed form):
      pav[q-tile, head, 0:65] += matmul(lhsT=expS[kv, q-tile], rhs=V2[kv, 0:65])
      accumulated over kv tiles; col 64 is the denominator.
    normalize on DVE (reciprocal + mult) -> attn [q, 256] fp16
    attn^T via PE transpose -> attnT [ch, q] (lhsT layout for out-proj)
  out-proj: y[t, e] = matmul(lhsT=attnT [256, t], rhs=Wo_c [256, e]) -> fp16

Emission is software-pipelined by hand because every engine executes its
queue in order: scores(i+1) is emitted before AV(i), out-proj blocks of the
previous q-chunk and the NEXT batch's projection chunks are spread into the
ACT-bound attention steps so the PE never parks on the exp it is about to
consume.
"""

import os

import numpy as np

import concourse.mybir as mybir
import concourse.tile as tile
from concourse import bacc
from concourse import bass_utils

P = 128
B = 2
T = 2048
C = 2048
HD = 64
QH = 32
KVH = 8
G = QH // KVH  # 4
NCORES = 8
TCH = 256  # token chunk for projection phase
QCH = 512  # q chunk for attention phase
KT = C // P  # 16 contraction tiles
NTCH = T // TCH  # 8
NQC = T // QCH  # 4
f32 = mybir.dt.float32
fp16 = mybir.dt.float16

_CACHE = {}


def _build():
    nc = bacc.Bacc("TRN2", target_bir_lowering=False, debug=False, num_devices=NCORES)

    xt = nc.dram_tensor("xt", [B, C, T], fp16, kind="ExternalInput")
    wqk = nc.dram_tensor("wqk", [C, 384], fp16, kind="ExternalInput")
    wv = nc.dram_tensor("wv", [C, HD], fp16, kind="ExternalInput")
    wo = nc.dram_tensor("wo", [G * HD, C], fp16, kind="ExternalInput")
    bqk = nc.dram_tensor("bqk", [P, 3], f32, kind="ExternalInput")
    bv = nc.dram_tensor("bv", [1, HD], f32, kind="ExternalInput")
    maskd = nc.dram_tensor("mask", [P, P], fp16, kind="ExternalInput")
    identd = nc.dram_tensor("ident", [P, P], fp16, kind="ExternalInput")
    y = nc.dram_tensor("y", [B, T, C], fp16, kind="ExternalOutput")

    wqk3 = wqk.ap().rearrange("(ko p) m -> p ko m", p=P)
    wo3 = wo.ap().rearrange("(ko p) m -> p ko m", p=P)

    with tile.TileContext(nc) as tc:
        with (
            tc.tile_pool(name="const", bufs=1) as cpool,
            tc.tile_pool(name="x", bufs=3) as xpool,
            tc.tile_pool(name="proj", bufs=2) as projpool,
            tc.tile_pool(name="v2p", bufs=2) as v2pool,
            tc.tile_pool(name="kk", bufs=2) as kkpool,
            tc.tile_pool(name="exps", bufs=32) as expool,
            tc.tile_pool(name="attnT", bufs=4) as apool,
            tc.tile_pool(name="attn", bufs=4) as aspool,
            tc.tile_pool(name="y", bufs=4) as ypool,
            tc.tile_pool(name="psS", bufs=2, space="PSUM") as psumS,
            tc.tile_pool(name="psAV", bufs=2, space="PSUM") as psumAV,
            tc.tile_pool(name="psY", bufs=2, space="PSUM") as psumY,
        ):
            # ---- constants / weights (resident) ----
            # startup-critical DMA order: first half of wqk sub0 + x chunk 0
            # (the first 8 QKK matmuls need only these), then the rest
            # cols 0:256 (both Q subs together) have 512B contiguous rows ->
            # full DMA rate; quarter-granularity first tiles cut the latency
            # to the very first matmul
            wqk_sb = cpool.tile([P, KT, 384], fp16)
            xb0 = xt.ap()[0].rearrange("(ko p) t -> p ko t", p=P)
            xch0 = xpool.tile([P, KT, TCH], fp16, tag="xch", name="xch")
            nc.sync.dma_start(wqk_sb[:, 0:4, :], wqk3[:, 0:4, :])
            nc.sync.dma_start(xch0[:, 0:4, :], xb0[:, 0:4, 0:TCH])
            nc.sync.dma_start(wqk_sb[:, 4:8, :], wqk3[:, 4:8, :])
            nc.sync.dma_start(xch0[:, 4:8, :], xb0[:, 4:8, 0:TCH])
            nc.sync.dma_start(wqk_sb[:, 8:KT, :], wqk3[:, 8:KT, :])
            nc.sync.dma_start(xch0[:, 8:KT, :], xb0[:, 8:KT, 0:TCH])
            xch1 = xpool.tile([P, KT, TCH], fp16, tag="xch", name="xch")
            nc.sync.dma_start(xch1[:, 0 : KT // 2, :], xb0[:, 0 : KT // 2, TCH : 2 * TCH])
            nc.sync.dma_start(xch1[:, KT // 2 :, :], xb0[:, KT // 2 :, TCH : 2 * TCH])
            bqk_sb = cpool.tile([P, 3], f32)
            nc.sync.dma_start(bqk_sb[:], bqk.ap())
            bv_sb = cpool.tile([P, HD], f32)
            nc.sync.dma_start(bv_sb[:], bv.ap().to_broadcast((P, HD)))
            mask_sb = cpool.tile([P, P], fp16)
            nc.sync.dma_start(mask_sb[:], maskd.ap())
            ident_sb = cpool.tile([P, P], fp16)
            nc.sync.dma_start(ident_sb[:], identd.ap())
            wo_sb = cpool.tile([P, 2, C], fp16)
            # PE warmup scratch: dummy matmuls keep the PE pstate ramped
            # through startup DMA waits (results are never read)
            scratch_sb = cpool.tile([P, QCH], fp16)
            nc.gpsimd.memset(scratch_sb[:], 0.0)

            def emit_dummy(n):
                for _ in range(n):
                    dmy = psumY.tile([P, QCH], f32, tag="py", name="dmy")
                    nc.tensor.matmul(
                        dmy[:],
                        scratch_sb[:, 0:P],
                        scratch_sb[:],
                        start=True,
                        stop=True,
                    )

            # ---- deferred-work queues (fill PE during ACT-bound stretches) --
            p3_queue = []  # out-proj (ts, ec) blocks of finished q-chunks
            p1_queue = []  # next batch's projection chunks
            p3_state = {}
            in_p1_phase = [True]  # psAV banks are free during projections

            def p3_block(pb, pattnT, tglob, ec):
                # one y row-block [128, C] per q-tile: 4 (ec) matmul+copy
                # units sharing a y_sb row, one big DMA after the last ec
                def emit():
                    if ec == 0:
                        p3_state[ts_key] = ypool.tile(
                            [P, C], fp16, tag="ysb", name="ysb"
                        )
                    y_sb = p3_state[ts_key]
                    if in_p1_phase[0] and (ec + tglob) % 2 == 0:
                        py = psumAV.tile([P, QCH], f32, tag="pav", name="py2")
                    else:
                        py = psumY.tile([P, QCH], f32, tag="py", name="py")
                    for ks in range(2):
                        nc.tensor.matmul(
                            py[:],
                            pattnT[:, ks, (tglob % 4) * P : (tglob % 4 + 1) * P],
                            wo_sb[:, ks, ec * QCH : (ec + 1) * QCH],
                            start=(ks == 0),
                            stop=(ks == 1),
                        )
                    last = pb == B - 1 and tglob >= 12
                    if last and ec % 2 == 1:
                        nc.scalar.copy(y_sb[:, ec * QCH : (ec + 1) * QCH], py[:])
                    else:
                        nc.vector.tensor_copy(y_sb[:, ec * QCH : (ec + 1) * QCH], py[:])
                    if last and tglob >= 14:
                        # tail latency: half-row DMAs via the fast HWDGE path,
                        # issued as soon as each half is copied
                        if ec == 1:
                            nc.sync.dma_start(
                                y.ap()[pb, tglob * P : (tglob + 1) * P, 0 : C // 2],
                                y_sb[:, 0 : C // 2],
                            )
                        elif ec == 3:
                            nc.sync.dma_start(
                                y.ap()[pb, tglob * P : (tglob + 1) * P, C // 2 :],
                                y_sb[:, C // 2 :],
                            )
                    elif ec == C // QCH - 1:
                        nc.gpsimd.dma_start(
                            y.ap()[pb, tglob * P : (tglob + 1) * P, :], y_sb[:]
                        )

                ts_key = (pb, tglob)
                return emit

            def queue_p3(pb, pattnT, pqc):
                for ts in range(QCH // P):
                    for ec in range(C // QCH):
                        p3_queue.append(p3_block(pb, pattnT, pqc * (QCH // P) + ts, ec))

            def pop_p3(n):
                for _ in range(min(n, len(p3_queue))):
                    p3_queue.pop(0)()

            # ---- P1: projection chunk emitters --------------------------
            def make_p1(b):
                xb = xt.ap()[b].rearrange("(ko p) t -> p ko t", p=P)
                qkk_sb = projpool.tile([P, 3, T], fp16, tag="qkk", name="qkk")
                kk_sb = kkpool.tile([P, T], fp16, tag="kk", name="kk")
                v2_sb = v2pool.tile([P, KT, HD + 1], fp16, tag="v2", name="v2")
                nc.gpsimd.memset(v2_sb[:, :, HD : HD + 1], 1.0)
                xchs = {}

                def load(tch):
                    tsl = slice(tch * TCH, (tch + 1) * TCH)
                    xch = xpool.tile([P, KT, TCH], fp16, tag="xch", name="xch")
                    nc.sync.dma_start(xch[:, 0 : KT // 2, :], xb[:, 0 : KT // 2, tsl])
                    nc.sync.dma_start(xch[:, KT // 2 :, :], xb[:, KT // 2 :, tsl])
                    xchs[tch] = xch

                def sub_proj(tch, sub):
                    # one ~1us unit: a single sub-projection chain
                    def emit():
                        if sub == 0:
                            if tch + 1 < NTCH and tch + 1 not in xchs:
                                load(tch + 1)
                            if b == 0 and tch == 4:
                                nc.sync.dma_start(wo_sb[:], wo3)
                        tsl = slice(tch * TCH, (tch + 1) * TCH)
                        xch = xchs[tch]
                        pp_full = psumY.tile([P, QCH], f32, tag="py", name="pp")
                        pp = pp_full[:, :TCH]
                        for k in range(KT):
                            nc.tensor.matmul(
                                pp[:],
                                wqk_sb[:, k, sub * P : (sub + 1) * P],
                                xch[:, k, :],
                                start=(k == 0),
                                stop=(k == KT - 1),
                            )
                        nc.vector.tensor_tensor(
                            qkk_sb[:, sub, tsl],
                            pp[:],
                            bqk_sb[:, sub : sub + 1].to_broadcast((P, TCH)),
                            mybir.AluOpType.add,
                        )

                    return emit

                def kv_finish(tch):
                    # K^T dup + V-natural transposes for one chunk
                    def emit():
                        tsl = slice(tch * TCH, (tch + 1) * TCH)
                        xchs.pop(tch, None)
                        nc.gpsimd.dma_start(
                            kk_sb[64:128, tsl], qkk_sb[0:64, 2, tsl]
                        )
                        for ts in range(TCH // P):
                            tidx = tch * (TCH // P) + ts
                            psl = slice(tidx * P, (tidx + 1) * P)
                            pv2 = psumY.tile([P, QCH], fp16, tag="py", name="pv2")
                            nc.tensor.transpose(
                                pv2[:, 0:HD],
                                qkk_sb[64:128, 2, psl],
                                ident_sb[64:128, 64:128],
                            )
                            nc.vector.tensor_tensor(
                                v2_sb[:, tidx, 0:HD],
                                pv2[:, 0:HD],
                                bv_sb[:],
                                mybir.AluOpType.add,
                            )

                    return emit

                if b == 0:
                    xchs[0] = xch0
                    xchs[1] = xch1
                else:
                    load(0)
                units = []
                for t in range(NTCH):
                    grp = [sub_proj(t, 0)]
                    if t > 0:
                        grp.append(kv_finish(t - 1))
                    grp += [sub_proj(t, 1), sub_proj(t, 2)]
                    units.append(grp)
                units.append([kv_finish(NTCH - 1)])
                return qkk_sb, kk_sb, v2_sb, units

            # ---- P2: attention for one batch ----------------------------
            def emit_p2(b, qkk_sb, kk_sb, v2_sb):
                for qc in range(NQC):
                    q0 = qc * QCH
                    nfull = q0 // P
                    ntiles = nfull + QCH // P

                    for sub in range(2):
                        extiles = {}
                        if sub == 0:
                            attnT = apool.tile(
                                [P, 2, QCH], fp16, tag="attnT", name="attnT"
                            )
                        pava = psumAV.tile(
                            [P, 2, 2, HD + 1], f32, tag="pav", name="pava"
                        )
                        pavb = psumAV.tile(
                            [P, 2, 2, HD + 1], f32, tag="pav", name="pavb"
                        )
                        pavs = (pava, pavb)

                        def emit_scores(i):
                            if i < nfull:
                                nsl = slice(0, QCH)
                            else:
                                nsl = slice((i - nfull) * P, QCH)
                            ksl = slice(i * P, (i + 1) * P)
                            ex = expool.tile([P, 2, QCH], fp16, tag="ex", name="ex")
                            extiles[i] = ex
                            ps_s = psumS.tile([P, 2, QCH], f32, tag="ps", name="ps_s")
                            # concurrent pair: disjoint PE rows 0-63 / 64-127
                            nc.tensor.matmul(
                                ps_s[:, 0, nsl],
                                qkk_sb[0:64, 2, ksl],
                                qkk_sb[0:64, sub, q0 + nsl.start : q0 + QCH],
                                start=True,
                                stop=True,
                            )
                            nc.tensor.matmul(
                                ps_s[:, 1, nsl],
                                kk_sb[64:128, ksl],
                                qkk_sb[64:128, sub, q0 + nsl.start : q0 + QCH],
                                start=True,
                                stop=True,
                            )
                            nc.scalar.activation(
                                ex[:, :, nsl],
                                ps_s[:, :, nsl],
                                mybir.ActivationFunctionType.Exp,
                                scale=0.125,
                            )
                            if i >= nfull:
                                j = i - nfull
                                nc.vector.tensor_tensor(
                                    ex[:, :, j * P : (j + 1) * P],
                                    ex[:, :, j * P : (j + 1) * P],
                                    mask_sb[:, None, :].to_broadcast((P, 2, P)),
                                    mybir.AluOpType.mult,
                                )

                        def emit_av(i):
                            # AV natural orientation, diagonal-restricted; the
                            # 4 q-tile chains x 2 heads share 2 PSUM banks:
                            # pav[ts%2, half, 0:65], col 64 = denominator.
                            for ts in range(QCH // P):
                                if nfull + ts < i:
                                    continue
                                pav = pavs[ts // 2]
                                for half in range(2):
                                    # start only on the FIRST matmul into each
                                    # PSUM bank: start_tensor_calc marks the
                                    # whole 2KB zero region pending-zero, so
                                    # each chain's first write self-initializes
                                    nc.tensor.matmul(
                                        pav[:, ts % 2, half, :],
                                        extiles[i][:, half, ts * P : (ts + 1) * P],
                                        v2_sb[:, i, :],
                                        start=(i == 0 and ts % 2 == 0 and half == 0),
                                        stop=(i == nfull + ts),
                                        skip_group_check=True,
                                    )
                            # chain ts = i - nfull just stopped: normalize it
                            ts = i - nfull
                            if 0 <= ts < QCH // P:
                                pav = pavs[ts // 2]
                                rec = aspool.tile([P, 2], f32, tag="rec", name="rec")
                                nc.vector.reciprocal(rec[:], pav[:, ts % 2, :, HD])
                                attn = aspool.tile(
                                    [P, 2, HD], fp16, tag="attn", name="attn"
                                )
                                nc.vector.tensor_tensor(
                                    attn[:],
                                    pav[:, ts % 2, :, 0:HD],
                                    rec[:, :, None].to_broadcast((P, 2, HD)),
                                    mybir.AluOpType.mult,
                                )
                                # attn^T via PE transpose: [128 q, 128 ch] ->
                                # attnT[p, sub, ts*P + q] with ch = sub*128 + p
                                ptr = psumY.tile([P, P], fp16, tag="py", name="ptr")
                                nc.tensor.transpose(
                                    ptr[:],
                                    attn[:].rearrange("p a b -> p (a b)"),
                                    ident_sb[:],
                                )
                                nc.vector.tensor_copy(
                                    attnT[:, sub, ts * P : (ts + 1) * P], ptr[:]
                                )
                                if b == B - 1 and qc == NQC - 1 and sub == 1:
                                    in_p1_phase[0] = True
                                    for ec in range(C // QCH):
                                        p3_block(
                                            b, attnT, qc * (QCH // P) + ts, ec
                                        )()
                                    in_p1_phase[0] = False

                        # lag-1 pipeline over kv tiles; drain the deferred
                        # queues evenly across this sub's steps
                        if sub == 0:
                            qc_popped[0] = 0
                        p3_backlog = len(p3_queue)
                        p3_target = p3_backlog // 2 if sub == 0 else 0
                        # in the last batch, hold back out-proj filler for the
                        # late (ACT-heavy) q-chunks
                        keep = ((6, 4, 0, 0), (8, 6, 0, 0))[b][qc]
                        p3_target = max(p3_target, min(keep, p3_backlog))
                        for i in range(ntiles + 2):
                            if i < ntiles:
                                emit_scores(i)
                            if i >= 2:
                                emit_av(i - 2)
                            want = p3_target + (
                                (p3_backlog - p3_target) * (ntiles + 1 - i)
                            ) // (ntiles + 2)
                            want = max(want, p3_target)
                            pop_p3(len(p3_queue) - want)
                            # spread the next batch's projection chunks.
                            # During P2(b0): 6 of the 8 chunks, evenly.
                            # During P2(b1): chunk tch is due before its own
                            # q-chunk (tch//2), so the last two drain there.
                            if p1_queue and b == 0:
                                quota = (4, 4, 8, 8)[qc]
                                k = sub * (ntiles + 2) + i + 1
                                steps_qc = 2 * (ntiles + 2)
                                if qc_popped[0] < quota and k * (quota + 1) >= (
                                    qc_popped[0] + 1
                                ) * steps_qc:
                                    p1_queue.pop(0)()
                                    qc_popped[0] += 1
                            elif p1_queue and b == 1 and sub == 0 and (
                                (qc < 2 and i in (2, 5, 8)) or (qc == 2 and i in (2, 5))
                            ):
                                p1_queue.pop(0)()
                        if sub == 1 and not (b == B - 1 and qc == NQC - 1):
                            queue_p3(b, attnT, qc)

            # ---- whole-kernel emission ----------------------------------
            qkk0, kk0, v20, chunks0 = make_p1(0)
            emit_dummy(4)
            for ci, ch in enumerate(chunks0):
                for u in ch:
                    u()
                pop_p3(2)
                if ci < 2:
                    emit_dummy(0)
            qkk1, kk1, v21, chunks1 = make_p1(1)
            for ch in chunks1:
                p1_queue.extend(ch)
            p1_done = []
            qc_popped = [0]
            in_p1_phase[0] = False
            emit_p2(0, qkk0, kk0, v20)
            in_p1_phase[0] = True
            while p1_queue:
                p1_queue.pop(0)()
            in_p1_phase[0] = False
            emit_p2(1, qkk1, kk1, v21)
            pop_p3(len(p3_queue))

    nc.compile()
    return nc


def _prep_inputs(x, Wq, bq, Wk, bk, Wv, bv, Wo, bo):
    x = np.ascontiguousarray(np.asarray(x, dtype=np.float32))
    xt = np.ascontiguousarray(x.transpose(0, 2, 1)).astype(np.float16)
    Wq = np.asarray(Wq, dtype=np.float32)
    Wk = np.asarray(Wk, dtype=np.float32)
    Wv = np.asarray(Wv, dtype=np.float32)
    Wo = np.asarray(Wo, dtype=np.float32)
    bq = np.asarray(bq, dtype=np.float32)
    bk = np.asarray(bk, dtype=np.float32)
    bv = np.asarray(bv, dtype=np.float32)

    # mask[kj, qi] = 1 iff kj <= qi  (upper triangular incl. diag)
    mask = np.triu(np.ones((P, P), dtype=np.float16)).copy()
    ident = np.eye(P, dtype=np.float16)
    in_maps = []
    for c in range(NCORES):
        qs = slice(c * G * HD, (c + 1) * G * HD)
        ks = slice(c * HD, (c + 1) * HD)
        wqk_c = np.concatenate([Wq[:, qs], Wk[:, ks], Wv[:, ks]], axis=1)
        bq_c = bq[qs]
        bqk_c = np.stack(
            [bq_c[0:128], bq_c[128:256], np.concatenate([bk[ks], 0 * bv[ks]])], axis=1
        )
        in_maps.append(
            {
                "xt": xt,
                "wqk": np.ascontiguousarray(wqk_c).astype(np.float16),
                "wv": np.ascontiguousarray(Wv[:, ks]).astype(np.float16),
                "wo": np.ascontiguousarray(Wo[qs, :]).astype(np.float16),
                "bqk": np.ascontiguousarray(bqk_c),
                "bv": np.ascontiguousarray(bv[None, ks]),
                "mask": mask,
                "ident": ident,
            }
        )
    return in_maps


def kernel(x, Wq, bq, Wk, bk, Wv, bv, Wo, bo, _trace=False):
    # NTFF tracing is unavailable through this axon client; make sure a
    # stray BASS_TRACE=1 in the environment cannot divert the run path.
    if not _trace:
        os.environ["BASS_NEVER_TRACE"] = "1"
    if "nc" not in _CACHE:
        _CACHE["nc"] = _build()
    nc = _CACHE["nc"]
    in_maps = _prep_inputs(x, Wq, bq, Wk, bk, Wv, bv, Wo, bo)
    res = bass_utils.run_bass_kernel_spmd(
        nc, in_maps, core_ids=list(range(NCORES)), trace=_trace
    )
    bo = np.asarray(bo, dtype=np.float32)
    y = np.zeros((B, T, C), dtype=np.float32)
    for c in range(NCORES):
        y += res.results[c]["y"].astype(np.float32)
    y += bo
    if _trace:
        return y, res
    return y


# revision 72
# speedup vs baseline: 1.4034x; 1.0021x over previous
"""GQA forward kernel for Trainium2, 8-core tensor-parallel (group-aligned).

Problem: B=2, T=2048, D=2048, 32 Q heads / 8 KV heads, head_dim 64, causal.

Sharding: core c owns KV head c and its 4 Q heads (whole GQA group), both
batches.  Output projection is row-parallel Megatron style: each core
contracts its 256 attention-output channels against its slice of Wo and the
host sums the 8 partial outputs (+ bo).

All device dataflow is fp16 (fp32 PSUM accumulation), which halves HBM
traffic vs fp32 and runs matmuls at 1 row/cycle at any tile width.

Per-core dataflow:
  x^T [C, T] fp16 (host-transposed)
    -> QKK proj: lhsT = [Wq_p0 | Wq_p1 | Wk | Wk] -> Q^T [256, T], K^T dup [128, T]
    -> V proj: natural orientation -> V2 [T, 65] (V plus ones col for the
       softmax denominator), per 128-token tile
  attention per (batch, q-chunk of 512):
    S^T[kv, q] = matmul(lhsT=K^T tile [64,128], rhs=Q^T [64, nsl]); the two
      heads of a pair run on disjoint PE row groups (base partitions 0 / 64)
    expS = ACT Exp(S^T / 8) -> SBUF fp16  (no max-subtraction: |S/8| <= ~6)
    causal: column-sliced matmuls + one triangle mask-mult on diagonal tiles
    AV in NATURAL orientation (half the PE cost of the transposed form):
      pav[q-tile, head, 0:65] += matmul(lhsT=expS[kv, q-tile], rhs=V2[kv, 0:65])
      accumulated over kv tiles; col 64 is the denominator.
    normalize on DVE (reciprocal + mult) -> attn [q, 256] fp16
    attn^T via PE transpose -> attnT [ch, q] (lhsT layout for out-proj)
  out-proj: y[t, e] = matmul(lhsT=attnT [256, t], rhs=Wo_c [256, e]) -> fp16

Emission is software-pipelined by hand because every engine executes its
queue in order: scores(i+1) is emitted before AV(i), out-proj blocks of the
previous q-chunk and the NEXT batch's projection chunks are spread into the
ACT-bound attention steps so the PE never parks on the exp it is about to
consume.
"""

import os

import numpy as np

import concourse.mybir as mybir
import concourse.tile as tile
from concourse import bacc
from concourse import bass_utils

P = 128
B = 2
T = 2048
C = 2048
HD = 64
QH = 32
KVH = 8
G = QH // KVH  # 4
NCORES = 8
TCH = 256  # token chunk for projection phase
QCH = 512  # q chunk for attention phase
KT = C // P  # 16 contraction tiles
NTCH = T // TCH  # 8
NQC = T // QCH  # 4
f32 = mybir.dt.float32
fp16 = mybir.dt.float16

_CACHE = {}


def _build():
    nc = bacc.Bacc("TRN2", target_bir_lowering=False, debug=False, num_devices=NCORES)

    xt = nc.dram_tensor("xt", [B, C, T], fp16, kind="ExternalInput")
    wqk = nc.dram_tensor("wqk", [C, 384], fp16, kind="ExternalInput")
    wv = nc.dram_tensor("wv", [C, HD], fp16, kind="ExternalInput")
    wo = nc.dram_tensor("wo", [G * HD, C], fp16, kind="ExternalInput")
    bqk = nc.dram_tensor("bqk", [P, 3], f32, kind="ExternalInput")
    bv = nc.dram_tensor("bv", [1, HD], f32, kind="ExternalInput")
    maskd = nc.dram_tensor("mask", [P, P], fp16, kind="ExternalInput")
    identd = nc.dram_tensor("ident", [P, P], fp16, kind="ExternalInput")
    y = nc.dram_tensor("y", [B, T, C], fp16, kind="ExternalOutput")

    wqk3 = wqk.ap().rearrange("(ko p) m -> p ko m", p=P)
    wo3 = wo.ap().rearrange("(ko p) m -> p ko m", p=P)

    with tile.TileContext(nc) as tc:
        with (
            tc.tile_pool(name="const", bufs=1) as cpool,
            tc.tile_pool(name="x", bufs=3) as xpool,
            tc.tile_pool(name="proj", bufs=2) as projpool,
            tc.tile_pool(name="v2p", bufs=2) as v2pool,
            tc.tile_pool(name="kk", bufs=2) as kkpool,
            tc.tile_pool(name="exps", bufs=32) as expool,
            tc.tile_pool(name="attnT", bufs=4) as apool,
            tc.tile_pool(name="attn", bufs=4) as aspool,
            tc.tile_pool(name="y", bufs=4) as ypool,
            tc.tile_pool(name="psS", bufs=2, space="PSUM") as psumS,
            tc.tile_pool(name="psAV", bufs=2, space="PSUM") as psumAV,
            tc.tile_pool(name="psY", bufs=2, space="PSUM") as psumY,
        ):
            # ---- constants / weights (resident) ----
            # startup-critical DMA order: first half of wqk sub0 + x chunk 0
            # (the first 8 QKK matmuls need only these), then the rest
            # cols 0:256 (both Q subs together) have 512B contiguous rows ->
            # full DMA rate; quarter-granularity first tiles cut the latency
            # to the very first matmul
            wqk_sb = cpool.tile([P, KT, 384], fp16)
            xb0 = xt.ap()[0].rearrange("(ko p) t -> p ko t", p=P)
            xch0 = xpool.tile([P, KT, TCH], fp16, tag="xch", name="xch")
            nc.sync.dma_start(wqk_sb[:, 0:4, :], wqk3[:, 0:4, :])
            nc.sync.dma_start(xch0[:, 0:4, :], xb0[:, 0:4, 0:TCH])
            nc.sync.dma_start(wqk_sb[:, 4:8, :], wqk3[:, 4:8, :])
            nc.sync.dma_start(xch0[:, 4:8, :], xb0[:, 4:8, 0:TCH])
            nc.sync.dma_start(wqk_sb[:, 8:KT, :], wqk3[:, 8:KT, :])
            nc.sync.dma_start(xch0[:, 8:KT, :], xb0[:, 8:KT, 0:TCH])
            xch1 = xpool.tile([P, KT, TCH], fp16, tag="xch", name="xch")
            nc.sync.dma_start(xch1[:, 0 : KT // 2, :], xb0[:, 0 : KT // 2, TCH : 2 * TCH])
            nc.sync.dma_start(xch1[:, KT // 2 :, :], xb0[:, KT // 2 :, TCH : 2 * TCH])
            bqk_sb = cpool.tile([P, 3], f32)
            nc.sync.dma_start(bqk_sb[:], bqk.ap())
            bv_sb = cpool.tile([P, HD], f32)
            nc.sync.dma_start(bv_sb[:], bv.ap().to_broadcast((P, HD)))
            mask_sb = cpool.tile([P, P], fp16)
            nc.sync.dma_start(mask_sb[:], maskd.ap())
            ident_sb = cpool.tile([P, P], fp16)
            nc.sync.dma_start(ident_sb[:], identd.ap())
            wo_sb = cpool.tile([P, 2, C], fp16)
            # PE warmup scratch: dummy matmuls keep the PE pstate ramped
            # through startup DMA waits (results are never read)
            scratch_sb = cpool.tile([P, QCH], fp16)
            nc.gpsimd.memset(scratch_sb[:], 0.0)

            def emit_dummy(n):
                for _ in range(n):
                    dmy = psumY.tile([P, QCH], f32, tag="py", name="dmy")
                    nc.tensor.matmul(
                        dmy[:],
                        scratch_sb[:, 0:P],
                        scratch_sb[:],
                        start=True,
                        stop=True,
                    )

            # ---- deferred-work queues (fill PE during ACT-bound stretches) --
            p3_queue = []  # out-proj (ts, ec) blocks of finished q-chunks
            p1_queue = []  # next batch's projection chunks
            p3_state = {}
            in_p1_phase = [True]  # psAV banks are free during projections

            def p3_block(pb, pattnT, tglob, ec):
                # one y row-block [128, C] per q-tile: 4 (ec) matmul+copy
                # units sharing a y_sb row, one big DMA after the last ec
                def emit():
                    if ec == 0:
                        p3_state[ts_key] = ypool.tile(
                            [P, C], fp16, tag="ysb", name="ysb"
                        )
                    y_sb = p3_state[ts_key]
                    if in_p1_phase[0] and (ec + tglob) % 2 == 0:
                        py = psumAV.tile([P, QCH], f32, tag="pav", name="py2")
                    else:
                        py = psumY.tile([P, QCH], f32, tag="py", name="py")
                    for ks in range(2):
                        nc.tensor.matmul(
                            py[:],
                            pattnT[:, ks, (tglob % 4) * P : (tglob % 4 + 1) * P],
                            wo_sb[:, ks, ec * QCH : (ec + 1) * QCH],
                            start=(ks == 0),
                            stop=(ks == 1),
                        )
                    last = pb == B - 1 and tglob >= 12
                    if last and ec % 2 == 1:
                        nc.scalar.copy(y_sb[:, ec * QCH : (ec + 1) * QCH], py[:])
                    else:
                        nc.vector.tensor_copy(y_sb[:, ec * QCH : (ec + 1) * QCH], py[:])
                    if last and tglob >= 14:
                        # tail latency: half-row DMAs via the fast HWDGE path,
                        # issued as soon as each half is copied
                        if ec == 1:
                            nc.sync.dma_start(
                                y.ap()[pb, tglob * P : (tglob + 1) * P, 0 : C // 2],
                                y_sb[:, 0 : C // 2],
                            )
                        elif ec == 3:
                            nc.sync.dma_start(
                                y.ap()[pb, tglob * P : (tglob + 1) * P, C // 2 :],
                                y_sb[:, C // 2 :],
                            )
                    elif ec == C // QCH - 1:
                        nc.gpsimd.dma_start(
                            y.ap()[pb, tglob * P : (tglob + 1) * P, :], y_sb[:]
                        )

                ts_key = (pb, tglob)
                return emit

            def queue_p3(pb, pattnT, pqc):
                for ts in range(QCH // P):
                    for ec in range(C // QCH):
                        p3_queue.append(p3_block(pb, pattnT, pqc * (QCH // P) + ts, ec))

            def pop_p3(n):
                for _ in range(min(n, len(p3_queue))):
                    p3_queue.pop(0)()

            # ---- P1: projection chunk emitters --------------------------
            def make_p1(b):
                xb = xt.ap()[b].rearrange("(ko p) t -> p ko t", p=P)
                qkk_sb = projpool.tile([P, 3, T], fp16, tag="qkk", name="qkk")
                kk_sb = kkpool.tile([P, T], fp16, tag="kk", name="kk")
                v2_sb = v2pool.tile([P, KT, HD + 1], fp16, tag="v2", name="v2")
                nc.gpsimd.memset(v2_sb[:, :, HD : HD + 1], 1.0)
                xchs = {}

                def load(tch):
                    tsl = slice(tch * TCH, (tch + 1) * TCH)
                    xch = xpool.tile([P, KT, TCH], fp16, tag="xch", name="xch")
                    nc.sync.dma_start(xch[:, 0 : KT // 2, :], xb[:, 0 : KT // 2, tsl])
                    nc.sync.dma_start(xch[:, KT // 2 :, :], xb[:, KT // 2 :, tsl])
                    xchs[tch] = xch

                def sub_proj(tch, sub):
                    # one ~1us unit: a single sub-projection chain
                    def emit():
                        if sub == 0:
                            if tch + 1 < NTCH and tch + 1 not in xchs:
                                load(tch + 1)
                            if b == 0 and tch == 4:
                                nc.sync.dma_start(wo_sb[:], wo3)
                        tsl = slice(tch * TCH, (tch + 1) * TCH)
                        xch = xchs[tch]
                        pp_full = psumY.tile([P, QCH], f32, tag="py", name="pp")
                        pp = pp_full[:, :TCH]
                        for k in range(KT):
                            nc.tensor.matmul(
                                pp[:],
                                wqk_sb[:, k, sub * P : (sub + 1) * P],
                                xch[:, k, :],
                                start=(k == 0),
                                stop=(k == KT - 1),
                            )
                        nc.vector.tensor_tensor(
                            qkk_sb[:, sub, tsl],
                            pp[:],
                            bqk_sb[:, sub : sub + 1].to_broadcast((P, TCH)),
                            mybir.AluOpType.add,
                        )

                    return emit

                def kv_finish(tch):
                    # K^T dup + V-natural transposes for one chunk
                    def emit():
                        tsl = slice(tch * TCH, (tch + 1) * TCH)
                        xchs.pop(tch, None)
                        nc.gpsimd.dma_start(
                            kk_sb[64:128, tsl], qkk_sb[0:64, 2, tsl]
                        )
                        for ts in range(TCH // P):
                            tidx = tch * (TCH // P) + ts
                            psl = slice(tidx * P, (tidx + 1) * P)
                            pv2 = psumY.tile([P, QCH], fp16, tag="py", name="pv2")
                            nc.tensor.transpose(
                                pv2[:, 0:HD],
                                qkk_sb[64:128, 2, psl],
                                ident_sb[64:128, 64:128],
                            )
                            nc.vector.tensor_tensor(
                                v2_sb[:, tidx, 0:HD],
                                pv2[:, 0:HD],
                                bv_sb[:],
                                mybir.AluOpType.add,
                            )

                    return emit

                if b == 0:
                    xchs[0] = xch0
                    xchs[1] = xch1
                else:
                    load(0)
                units = []
                for t in range(NTCH):
                    grp = [sub_proj(t, 0)]
                    if t > 0:
                        grp.append(kv_finish(t - 1))
                    grp += [sub_proj(t, 1), sub_proj(t, 2)]
                    units.append(grp)
                units.append([kv_finish(NTCH - 1)])
                return qkk_sb, kk_sb, v2_sb, units

            # ---- P2: attention for one batch ----------------------------
            def emit_p2(b, qkk_sb, kk_sb, v2_sb):
                for qc in range(NQC):
                    q0 = qc * QCH
                    nfull = q0 // P
                    ntiles = nfull + QCH // P

                    for sub in range(2):
                        extiles = {}
                        if sub == 0:
                            attnT = apool.tile(
                                [P, 2, QCH], fp16, tag="attnT", name="attnT"
                            )
                        pava = psumAV.tile(
                            [P, 2, 2, HD + 1], f32, tag="pav", name="pava"
                        )
                        pavb = psumAV.tile(
                            [P, 2, 2, HD + 1], f32, tag="pav", name="pavb"
                        )
                        pavs = (pava, pavb)

                        def emit_scores(i):
                            if i < nfull:
                                nsl = slice(0, QCH)
                            else:
                                nsl = slice((i - nfull) * P, QCH)
                            ksl = slice(i * P, (i + 1) * P)
                            ex = expool.tile([P, 2, QCH], fp16, tag="ex", name="ex")
                            extiles[i] = ex
                            ps_s = psumS.tile([P, 2, QCH], f32, tag="ps", name="ps_s")
                            # concurrent pair: disjoint PE rows 0-63 / 64-127
                            nc.tensor.matmul(
                                ps_s[:, 0, nsl],
                                qkk_sb[0:64, 2, ksl],
                                qkk_sb[0:64, sub, q0 + nsl.start : q0 + QCH],
                                start=True,
                                stop=True,
                            )
                            nc.tensor.matmul(
                                ps_s[:, 1, nsl],
                                kk_sb[64:128, ksl],
                                qkk_sb[64:128, sub, q0 + nsl.start : q0 + QCH],
                                start=True,
                                stop=True,
                            )
                            nc.scalar.activation(
                                ex[:, :, nsl],
                                ps_s[:, :, nsl],
                                mybir.ActivationFunctionType.Exp,
                                scale=0.125,
                            )
                            if i >= nfull:
                                j = i - nfull
                                nc.vector.tensor_tensor(
                                    ex[:, :, j * P : (j + 1) * P],
                                    ex[:, :, j * P : (j + 1) * P],
                                    mask_sb[:, None, :].to_broadcast((P, 2, P)),
                                    mybir.AluOpType.mult,
                                )

                        def emit_av(i):
                            # AV natural orientation, diagonal-restricted; the
                            # 4 q-tile chains x 2 heads share 2 PSUM banks:
                            # pav[ts%2, half, 0:65], col 64 = denominator.
                            for ts in range(QCH // P):
                                if nfull + ts < i:
                                    continue
                                pav = pavs[ts // 2]
                                for half in range(2):
                                    # start only on the FIRST matmul into each
                                    # PSUM bank: start_tensor_calc marks the
                                    # whole 2KB zero region pending-zero, so
                                    # each chain's first write self-initializes
                                    nc.tensor.matmul(
                                        pav[:, ts % 2, half, :],
                                        extiles[i][:, half, ts * P : (ts + 1) * P],
                                        v2_sb[:, i, :],
                                        start=(i == 0 and ts % 2 == 0 and half == 0),
                                        stop=(i == nfull + ts),
                                        skip_group_check=True,
                                    )
                            # chain ts stopped LAST step: normalizing one
                            # step late gives DVE slack before the PE hits
                            # the transpose that consumes its output
                            ts = i - nfull - 1
                            if 0 <= ts < QCH // P:
                                pav = pavs[ts // 2]
                                rec = aspool.tile([P, 2], f32, tag="rec", name="rec")
                                nc.vector.reciprocal(rec[:], pav[:, ts % 2, :, HD])
                                attn = aspool.tile(
                                    [P, 2, HD], fp16, tag="attn", name="attn"
                                )
                                nc.vector.tensor_tensor(
                                    attn[:],
                                    pav[:, ts % 2, :, 0:HD],
                                    rec[:, :, None].to_broadcast((P, 2, HD)),
                                    mybir.AluOpType.mult,
                                )
                                # attn^T via PE transpose: [128 q, 128 ch] ->
                                # attnT[p, sub, ts*P + q] with ch = sub*128 + p
                                ptr = psumY.tile([P, P], fp16, tag="py", name="ptr")
                                nc.tensor.transpose(
                                    ptr[:],
                                    attn[:].rearrange("p a b -> p (a b)"),
                                    ident_sb[:],
                                )
                                nc.vector.tensor_copy(
                                    attnT[:, sub, ts * P : (ts + 1) * P], ptr[:]
                                )
                                if b == B - 1 and qc == NQC - 1 and sub == 1:
                                    in_p1_phase[0] = True
                                    for ec in range(C // QCH):
                                        p3_block(
                                            b, attnT, qc * (QCH // P) + ts, ec
                                        )()
                                    in_p1_phase[0] = False

                        # lag-1 pipeline over kv tiles; drain the deferred
                        # queues evenly across this sub's steps
                        if sub == 0:
                            qc_popped[0] = 0
                        p3_backlog = len(p3_queue)
                        p3_target = p3_backlog // 2 if sub == 0 else 0
                        # in the last batch, hold back out-proj filler for the
                        # late (ACT-heavy) q-chunks
                        keep = ((6, 4, 0, 0), (8, 6, 0, 0))[b][qc]
                        p3_target = max(p3_target, min(keep, p3_backlog))
                        for i in range(ntiles + 3):
                            if i < ntiles:
                                emit_scores(i)
                            if 2 <= i <= ntiles + 2:
                                emit_av(i - 2)
                            want = p3_target + (
                                (p3_backlog - p3_target) * (ntiles + 1 - i)
                            ) // (ntiles + 2)
                            want = max(want, p3_target)
                            pop_p3(len(p3_queue) - want)
                            # spread the next batch's projection chunks.
                            # During P2(b0): 6 of the 8 chunks, evenly.
                            # During P2(b1): chunk tch is due before its own
                            # q-chunk (tch//2), so the last two drain there.
                            if p1_queue and b == 0:
                                quota = (4, 4, 8, 8)[qc]
                                k = sub * (ntiles + 2) + i + 1
                                steps_qc = 2 * (ntiles + 2)
                                if qc_popped[0] < quota and k * (quota + 1) >= (
                                    qc_popped[0] + 1
                                ) * steps_qc:
                                    p1_queue.pop(0)()
                                    qc_popped[0] += 1
                            elif p1_queue and b == 1 and sub == 0 and (
                                (qc < 2 and i in (2, 5, 8)) or (qc == 2 and i in (2, 5))
                            ):
                                p1_queue.pop(0)()
                        if sub == 1 and not (b == B - 1 and qc == NQC - 1):
                            queue_p3(b, attnT, qc)

            # ---- whole-kernel emission ----------------------------------
            qkk0, kk0, v20, chunks0 = make_p1(0)
            emit_dummy(4)
            for ci, ch in enumerate(chunks0):
                for u in ch:
                    u()
                pop_p3(2)
                if ci < 2:
                    emit_dummy(0)
            qkk1, kk1, v21, chunks1 = make_p1(1)
            for ch in chunks1:
                p1_queue.extend(ch)
            p1_done = []
            qc_popped = [0]
            in_p1_phase[0] = False
            emit_p2(0, qkk0, kk0, v20)
            in_p1_phase[0] = True
            while p1_queue:
                p1_queue.pop(0)()
            in_p1_phase[0] = False
            emit_p2(1, qkk1, kk1, v21)
            pop_p3(len(p3_queue))

    nc.compile()
    return nc


def _prep_inputs(x, Wq, bq, Wk, bk, Wv, bv, Wo, bo):
    x = np.ascontiguousarray(np.asarray(x, dtype=np.float32))
    xt = np.ascontiguousarray(x.transpose(0, 2, 1)).astype(np.float16)
    Wq = np.asarray(Wq, dtype=np.float32)
    Wk = np.asarray(Wk, dtype=np.float32)
    Wv = np.asarray(Wv, dtype=np.float32)
    Wo = np.asarray(Wo, dtype=np.float32)
    bq = np.asarray(bq, dtype=np.float32)
    bk = np.asarray(bk, dtype=np.float32)
    bv = np.asarray(bv, dtype=np.float32)

    # mask[kj, qi] = 1 iff kj <= qi  (upper triangular incl. diag)
    mask = np.triu(np.ones((P, P), dtype=np.float16)).copy()
    ident = np.eye(P, dtype=np.float16)
    in_maps = []
    for c in range(NCORES):
        qs = slice(c * G * HD, (c + 1) * G * HD)
        ks = slice(c * HD, (c + 1) * HD)
        wqk_c = np.concatenate([Wq[:, qs], Wk[:, ks], Wv[:, ks]], axis=1)
        bq_c = bq[qs]
        bqk_c = np.stack(
            [bq_c[0:128], bq_c[128:256], np.concatenate([bk[ks], 0 * bv[ks]])], axis=1
        )
        in_maps.append(
            {
                "xt": xt,
                "wqk": np.ascontiguousarray(wqk_c).astype(np.float16),
                "wv": np.ascontiguousarray(Wv[:, ks]).astype(np.float16),
                "wo": np.ascontiguousarray(Wo[qs, :]).astype(np.float16),
                "bqk": np.ascontiguousarray(bqk_c),
                "bv": np.ascontiguousarray(bv[None, ks]),
                "mask": mask,
                "ident": ident,
            }
        )
    return in_maps


def kernel(x, Wq, bq, Wk, bk, Wv, bv, Wo, bo, _trace=False):
    # NTFF tracing is unavailable through this axon client; make sure a
    # stray BASS_TRACE=1 in the environment cannot divert the run path.
    if not _trace:
        os.environ["BASS_NEVER_TRACE"] = "1"
    if "nc" not in _CACHE:
        _CACHE["nc"] = _build()
    nc = _CACHE["nc"]
    in_maps = _prep_inputs(x, Wq, bq, Wk, bk, Wv, bv, Wo, bo)
    res = bass_utils.run_bass_kernel_spmd(
        nc, in_maps, core_ids=list(range(NCORES)), trace=_trace
    )
    bo = np.asarray(bo, dtype=np.float32)
    y = np.zeros((B, T, C), dtype=np.float32)
    for c in range(NCORES):
        y += res.results[c]["y"].astype(np.float32)
    y += bo
    if _trace:
        return y, res
    return y
